# revision 22
# baseline (speedup 1.0000x reference)
"""Trainium2 Bass kernel: per-channel circular conv via DFT matmuls, summed
over channels (sparse PSF kernel), 8-core channel-sharded SPMD.

out[b] = irfft2( sum_c rfft2(x[b,c]) * rfft2(scatter(relu(vk), idx)[c]) )

Sharding: each core owns 4 of 32 channels (forward FFTs + pointwise
multiply-accumulate), ReduceScatter(add) over batch gives core b the summed
spectrum of batch b, which it inverse-transforms. All FFTs are dense DFT
matmuls in float32r (full PE rate at moving-dim >= 256, even N required).

Spectra are kept transposed ("T-form", [q (0..256) x j (0..511)]) with the
m>256 half stored conjugated at its natural compute position ("P-form") so
no data reversal is ever needed - all permutations/conjugations/signs are
absorbed into host-precomputed constant matrices, including the inverse.
"""
import numpy as np

N = 512
NQ = 257
NE = 258          # even-padded 257 (fp32r matmul needs even moving dim)
NB = 8            # batches (one per core after reduce-scatter)
CL = 4            # channels per core
NC_TOT = 32
NCORES = 8
TH = 2 * np.pi / N
PB = 2 * 2 * N * 128 + 2 * N   # per-batch rs payload: 2 qchunks x 2 planes + nyq r/i
NW = 514          # even-aligned pointwise layout: a=[0:258), b=[258:514)

_CACHE = {}


def _consts():
    r = np.arange(N)
    m = np.arange(NQ)
    ang1 = TH * np.outer(r, m)
    FrT = np.zeros((N, NE), np.float32)
    FiT = np.zeros((N, NE), np.float32)
    FrT[:, :NQ] = np.cos(ang1)
    FiT[:, :NQ] = -np.sin(ang1)
    q = np.arange(256)
    ang2 = TH * np.outer(r, q)
    GrT = np.cos(ang2).astype(np.float32)
    GiT = (-np.sin(ang2)).astype(np.float32)
    GnT = -GiT
    altT = ((-1.0) ** r).astype(np.float32).reshape(N, 1)
    w = np.full(NQ, 2.0)
    w[0] = 1.0
    w[256] = 1.0
    angA = TH * np.outer(np.arange(NQ), r)
    Acos = (w[:, None] * np.cos(angA)).astype(np.float32)
    Asin = (w[:, None] * np.sin(angA)).astype(np.float32)
    Ansin = -Asin
    j = np.arange(N)
    angB = TH * np.outer(j, r)
    sgn = np.ones((N, N))
    sgn[257:, :] = ((-1.0) ** r)[None, :]
    Bcos_t = (np.cos(angB) * sgn / (N * N)).astype(np.float32)
    Bsin_t = (-np.sin(angB) * sgn / (N * N)).astype(np.float32)

    def bpack(Bm):
        out = np.zeros((640, N), np.float32)
        out[0:128] = Bm[0:128]
        out[128:256] = Bm[128:256]
        out[256:256 + 127] = Bm[257:384]
        out[384:512] = Bm[384:512]
        out[512:513] = Bm[256:257]
        return out
    Bcos = bpack(Bcos_t)
    Bsin = bpack(Bsin_t)
    ones4 = np.ones((CL, 1), np.float32)
    return dict(FrT=FrT, FiT=FiT, GrT=GrT, GiT=GiT, GnT=GnT, altT=altT,
                Acos=Acos, Asin=Asin, Ansin=Ansin, Bcos=Bcos, Bsin=Bsin,
                ones4=ones4)


CONST_SHAPES = dict(FrT=(N, NE), FiT=(N, NE), GrT=(N, 256), GiT=(N, 256),
                    GnT=(N, 256), altT=(N, 1), Acos=(NQ, N), Asin=(NQ, N),
                    Ansin=(NQ, N), Bcos=(640, N), Bsin=(640, N), ones4=(CL, 1))
# consts carried in bf16 when the kernel runs its bf16 spectral-domain path
CONST_BF = {"Acos", "Asin", "Ansin", "Bcos", "Bsin"}


def _build_nc(repeat=1, variant="full"):
    import concourse.bacc as bacc
    import concourse.mybir as mybir
    import concourse.tile as tile

    f32 = mybir.dt.float32
    f32r = mybir.dt.float32r
    mult = mybir.AluOpType.mult
    add = mybir.AluOpType.add
    sub = mybir.AluOpType.subtract

    PQ = variant in ("full", "pq", "no_rs", "no_pw")
    BF = variant in ("full", "no_rs", "no_pw")
    bf16 = mybir.dt.bfloat16
    # pointwise / rs / inverse data dtype: uniform end-to-end so no DMA ever
    # needs a cast (f32r is plain f32 storage)
    kdt = bf16 if BF else (f32r if PQ else f32)
    cdt = f32r                      # inverse-const dtype (inverse stays f32r)

    nc = bacc.Bacc("TRN2", target_bir_lowering=False, debug=False,
                   enable_asserts=False, num_devices=NCORES)
    xs_in = nc.dram_tensor("xs", [NB * CL, N, N], f32r, kind="ExternalInput")
    kd_in = nc.dram_tensor("kd", [CL, N, N], f32r, kind="ExternalInput")
    cins = {nm: nc.dram_tensor(nm, list(sh),
                               cdt if nm in CONST_BF else f32r,
                               kind="ExternalInput")
            for nm, sh in CONST_SHAPES.items()}
    y_out = nc.dram_tensor("y", [N, N], f32, kind="ExternalOutput")

    xio_bufs = 2 if (PQ and not BF) else 3
    crt_bufs = 4 if (PQ and not BF) else 6
    with tile.TileContext(nc) as tc:
        with tc.tile_pool(name="consts", bufs=1) as cp, \
             tc.tile_pool(name="kf", bufs=1) as kp, \
             tc.tile_pool(name="xio", bufs=xio_bufs) as xp, \
             tc.tile_pool(name="crt", bufs=crt_bufs) as crp, \
             tc.tile_pool(name="acc", bufs=2) as ap, \
             tc.tile_pool(name="tmp", bufs=2) as tp, \
             tc.tile_pool(name="tmp2", bufs=2) as tp2, \
             tc.tile_pool(name="inv", bufs=1) as ivp, \
             tc.tile_pool(name="psA", bufs=2, space="PSUM") as psA, \
             tc.tile_pool(name="psB", bufs=4, space="PSUM") as psB, \
             tc.tile_pool(name="psN", bufs=2, space="PSUM") as psN, \
             tc.tile_pool(name="dram", bufs=1, space="DRAM") as dp:

            # ---- load constants (chunked along partition) ----
            def load_const(nm, rows, cols):
                ts = []
                dt = cdt if nm in CONST_BF else f32r
                nch = (rows + 127) // 128
                for k in range(nch):
                    p = min(128, rows - k * 128)
                    t = cp.tile([p, cols], dt, name=f"{nm}{k}", tag=f"{nm}{k}")
                    nc.sync.dma_start(t[:], cins[nm][k * 128:k * 128 + p, :])
                    ts.append(t)
                return ts

            Fr = load_const("FrT", N, NE)
            Fi = load_const("FiT", N, NE)
            Gr = load_const("GrT", N, 256)
            Gi = load_const("GiT", N, 256)
            Gn = load_const("GnT", N, 256)
            alt = load_const("altT", N, 1)
            Ac = load_const("Acos", NQ, N)   # chunks: 128,128,1
            As = load_const("Asin", NQ, N)
            An = load_const("Ansin", NQ, N)
            Bc = load_const("Bcos", 640, N)
            Bs = load_const("Bsin", 640, N)
            o4 = load_const("ones4", CL, 1)

            # ---- persistent Kf storage ----
            KW = NW if PQ else N
            kfr = [[kp.tile([128, KW], kdt, name=f"kfr{c}_{qc}", tag=f"kfr{c}_{qc}") for qc in range(2)]
                   for c in range(CL)]
            kfi = [[kp.tile([128, KW], kdt, name=f"kfi{c}_{qc}", tag=f"kfi{c}_{qc}") for qc in range(2)]
                   for c in range(CL)]
            kfnr = kp.tile([CL, NE], f32, name="kfnr", tag="kfnr")   # nyq strips packed by channel
            kfni = kp.tile([CL, NE], f32, name="kfni", tag="kfni")

            def consume_pq(mode, c, qc, P, Q, U, V, acc):
                """ra=P+Q  rb=P-Q  ia=U-V  ib=U+V.  Layout (width NW=514):
                cols [0:258)=a-region (j 0..256 + zero pad), [258:514)=b-region
                (j 257..511 from psum cols 1..256, col 513 = pad).  All
                offsets/widths even: DVE f32->bf16 writes at odd offsets are
                silently corrupt (probed), so the odd 257-col split is out."""
                def combine(dst, x, y, op_a, op_b):
                    nc.vector.tensor_tensor(dst[:, 0:NE], x[:, 0:NE], y[:, 0:NE], op=op_a)
                    nc.vector.tensor_tensor(dst[:, NE:NW], x[:, 1:NQ], y[:, 1:NQ], op=op_b)
                if mode == "k":
                    combine(kfr[c][qc], P, Q, add, sub)
                    combine(kfi[c][qc], U, V, sub, add)
                    return
                if variant == "no_pw":
                    return
                rAB = tp.tile([128, NW], kdt, name="rAB", tag="rAB")
                iAB = tp.tile([128, NW], kdt, name="iAB", tag="iAB")
                combine(rAB, P, Q, add, sub)
                combine(iAB, U, V, sub, add)
                kr, ki = kfr[c][qc], kfi[c][qc]
                t1 = tp.tile([128, NW], kdt, name="t1", tag="t1")
                t2 = tp.tile([128, NW], kdt, name="t2", tag="t2")
                base = qc * 2 * NW
                ar = acc[:, base:base + NW]
                ai = acc[:, base + NW:base + 2 * NW]
                nc.vector.tensor_tensor(t1[:], rAB[:], kr[:], op=mult)
                nc.vector.tensor_tensor(t2[:], iAB[:], ki[:], op=mult)
                if c == 0:
                    nc.vector.tensor_tensor(ar, t1[:], t2[:], op=sub)
                else:
                    nc.vector.tensor_tensor(t1[:], t1[:], t2[:], op=sub)
                    nc.vector.tensor_tensor(ar, ar, t1[:], op=add)
                nc.vector.tensor_tensor(t1[:], rAB[:], ki[:], op=mult)
                nc.vector.tensor_tensor(t2[:], iAB[:], kr[:], op=mult)
                if c == 0:
                    nc.vector.tensor_tensor(ai, t1[:], t2[:], op=add)
                else:
                    nc.vector.tensor_tensor(t1[:], t1[:], t2[:], op=add)
                    nc.vector.tensor_tensor(ai, ai, t1[:], op=add)

            def consume_pair(mode, c, qc, pR, pI, region, acc):
                """pR/pI: the psum pair for this region. region 'a': output
                cols [0:257]; 'b': cols [257:512] (psum cols [1:256])."""
                if region == "a":
                    cols_p, cols_a = slice(0, NQ), slice(0, NQ)
                else:
                    cols_p, cols_a = slice(1, 256), slice(NQ, N)
                w = cols_p.stop - cols_p.start
                if variant == "no_pw" and mode == "x":
                    return
                if mode == "k":
                    nc.scalar.copy(kfr[c][qc][:, cols_a], pR[:, cols_p])
                    nc.scalar.copy(kfi[c][qc][:, cols_a], pI[:, cols_p])
                    return
                kr = kfr[c][qc][:, cols_a]
                ki = kfi[c][qc][:, cols_a]
                # DVE: products straight from PSUM; combines on Pool unless
                # pw_dve (keeps the Pool queue free for the collective).
                ce = nc.vector if variant == "pw_dve" else nc.gpsimd
                t1 = tp.tile([128, NQ], f32, name="t1", tag="t1")
                t2 = tp.tile([128, NQ], f32, name="t2", tag="t2")
                t3 = tp.tile([128, NQ], f32, name="t3", tag="t3")
                t4 = tp.tile([128, NQ], f32, name="t4", tag="t4")
                nc.vector.tensor_tensor(t1[:, :w], pR[:, cols_p], kr, op=mult)
                nc.vector.tensor_tensor(t2[:, :w], pI[:, cols_p], ki, op=mult)
                nc.vector.tensor_tensor(t3[:, :w], pR[:, cols_p], ki, op=mult)
                nc.vector.tensor_tensor(t4[:, :w], pI[:, cols_p], kr, op=mult)
                base = qc * 2 * N
                ar = acc[:, base + cols_a.start: base + cols_a.stop]
                ai = acc[:, base + N + cols_a.start: base + N + cols_a.stop]
                if c == 0:
                    ce.tensor_tensor(ar, t1[:, :w], t2[:, :w], op=sub)
                    ce.tensor_tensor(ai, t3[:, :w], t4[:, :w], op=add)
                else:
                    ce.tensor_tensor(t1[:, :w], t1[:, :w], t2[:, :w], op=sub)
                    ce.tensor_tensor(ar, ar, t1[:, :w], op=add)
                    ce.tensor_tensor(t3[:, :w], t3[:, :w], t4[:, :w], op=add)
                    ce.tensor_tensor(ai, ai, t3[:, :w], op=add)

            def forward_image(src, mode, c, acc=None, xnyq=None):
                """src: DRAM AP [N, N]. mode 'k' fills kf tiles for channel c;
                mode 'x' pointwise-accumulates vs kf into the acc tile."""
                xt = xp.tile([128, 4 * N], f32r, name="xt", tag="xt")
                nc.sync.dma_start(xt.rearrange("p (k f) -> p k f", k=4),
                                  src.rearrange("(k p) f -> p k f", k=4))
                # stage 1: C^T[n, m] (m in [0,257), col 257 zero)
                crt, cit = [], []
                for n in range(4):
                    pr = psA.tile([128, NE], f32, name="ps1r", tag="ps1")
                    pi = psA.tile([128, NE], f32, name="ps1i", tag="ps1")
                    for k in range(4):
                        lhs = xt[:, k * N + n * 128:k * N + (n + 1) * 128]
                        nc.tensor.matmul(pr[:], lhsT=lhs, rhs=Fr[k][:],
                                         start=(k == 0), stop=(k == 3))
                        nc.tensor.matmul(pi[:], lhsT=lhs, rhs=Fi[k][:],
                                         start=(k == 0), stop=(k == 3))
                    cr = crp.tile([128, NE], f32r, name="cr", tag="cr")
                    ci = crp.tile([128, NE], f32r, name="ci", tag="ci")
                    nc.scalar.copy(cr[:], pr[:])
                    nc.scalar.copy(ci[:], pi[:])
                    crt.append(cr)
                    cit.append(ci)

                # stage 2 per q-chunk.  PQ path: with Gn = -Gi the four
                # region outputs are ra=P+Q, rb=P-Q, ia=U-V, ib=U+V from just
                # four matmul chains (P=Gr.crt, Q=Gn.cit, U=Gr.cit, V=Gn.crt)
                # - half the matmuls, and each loaded weight feeds 2 MMs.
                if PQ:
                    for qc in range(2):
                        qs = slice(qc * 128, (qc + 1) * 128)
                        P = psB.tile([128, NE], f32, name="Pp", tag="ps2")
                        Q = psB.tile([128, NE], f32, name="Qp", tag="ps2")
                        U = psB.tile([128, NE], f32, name="Up", tag="ps2")
                        V = psB.tile([128, NE], f32, name="Vp", tag="ps2")
                        for k in range(4):
                            st, sp = (k == 0), (k == 3)
                            nc.tensor.matmul(P[:], lhsT=Gr[k][:, qs], rhs=crt[k][:], start=st, stop=sp)
                            nc.tensor.matmul(U[:], lhsT=Gr[k][:, qs], rhs=cit[k][:], start=st, stop=sp)
                            nc.tensor.matmul(Q[:], lhsT=Gn[k][:, qs], rhs=cit[k][:], start=st, stop=sp)
                            nc.tensor.matmul(V[:], lhsT=Gn[k][:, qs], rhs=crt[k][:], start=st, stop=sp)
                        # DVE can read at most one PSUM operand per op: stage
                        # Q/V through SBUF (ACT), combine P/U straight from PSUM
                        Qs = tp.tile([128, NE], f32, name="Qs", tag="Qs")
                        Vs = tp.tile([128, NE], f32, name="Vs", tag="Vs")
                        nc.scalar.copy(Qs[:], Q[:])
                        nc.scalar.copy(Vs[:], V[:])
                        consume_pq(mode, c, qc, P, Qs, U, Vs, acc)
                else:
                    for qc in range(2):
                        qs = slice(qc * 128, (qc + 1) * 128)
                        a_s = slice(0, NE)
                        b_s = slice(0, 256)
                        ra = psB.tile([128, NE], f32, name="ra", tag="ps2")
                        ia = psB.tile([128, NE], f32, name="ia", tag="ps2")
                        for k in range(4):
                            st, sp = (k == 0), (k == 3)
                            nc.tensor.matmul(ra[:], lhsT=Gr[k][:, qs], rhs=crt[k][:, a_s], start=st, stop=False)
                            nc.tensor.matmul(ia[:], lhsT=Gr[k][:, qs], rhs=cit[k][:, a_s], start=st, stop=False)
                            nc.tensor.matmul(ra[:], lhsT=Gn[k][:, qs], rhs=cit[k][:, a_s], start=False, stop=sp)
                            nc.tensor.matmul(ia[:], lhsT=Gi[k][:, qs], rhs=crt[k][:, a_s], start=False, stop=sp)
                        consume_pair(mode, c, qc, ra, ia, "a", acc)
                        rb = psB.tile([128, 256], f32, name="rb", tag="ps2")
                        ib = psB.tile([128, 256], f32, name="ib", tag="ps2")
                        for k in range(4):
                            st, sp = (k == 0), (k == 3)
                            nc.tensor.matmul(rb[:], lhsT=Gr[k][:, qs], rhs=crt[k][:, b_s], start=st, stop=False)
                            nc.tensor.matmul(ib[:], lhsT=Gr[k][:, qs], rhs=cit[k][:, b_s], start=st, stop=False)
                            nc.tensor.matmul(rb[:], lhsT=Gi[k][:, qs], rhs=cit[k][:, b_s], start=False, stop=sp)
                            nc.tensor.matmul(ib[:], lhsT=Gn[k][:, qs], rhs=crt[k][:, b_s], start=False, stop=sp)
                        consume_pair(mode, c, qc, rb, ib, "b", acc)

                # nyquist strip q=256 (j in [0,258))
                nr = psN.tile([1, NE], f32, name="nr", tag="psn")
                ni = psN.tile([1, NE], f32, name="ni", tag="psn")
                for k in range(4):
                    st, sp = (k == 0), (k == 3)
                    nc.tensor.matmul(nr[:], lhsT=alt[k][:], rhs=crt[k][:], start=st, stop=sp)
                    nc.tensor.matmul(ni[:], lhsT=alt[k][:], rhs=cit[k][:], start=st, stop=sp)
                sr = tp2.tile([1, NE], f32, name="nstr", tag="nstr")
                si = tp2.tile([1, NE], f32, name="nsti", tag="nstr")
                nc.scalar.copy(sr[:], nr[:])
                nc.scalar.copy(si[:], ni[:])
                dst = (kfnr, kfni) if mode == "k" else xnyq
                nc.sync.dma_start(dst[0][c:c + 1, :], sr[:])
                nc.sync.dma_start(dst[1][c:c + 1, :], si[:])

            # double-buffered collective staging so RS(i) overlaps compute(i+1)
            # PQ ships the padded NW layout verbatim (RS is elementwise, any
            # consistent layout sums fine); the inverse unpacks it.
            PBv = (4 * 128 * NW + 2 * N) if PQ else PB
            rs_in = [dp.tile([NCORES, PBv], kdt, name=f"rs_in{p}", tag=f"rs_in{p}")
                     for p in range(2)]
            rs_out = [dp.tile([PBv], kdt, name=f"rs_out{p}", tag=f"rs_out{p}")
                      for p in range(2)]

            def inverse(rs_o):
                if variant == "no_rs":
                    rs_o = rs_in[0][0]
                idt = f32r
                dmae = nc.gpsimd
                Tr = [ivp.tile([128, N], idt, name=f"Tr{qc}", tag=f"Tr{qc}") for qc in range(2)]
                Ti = [ivp.tile([128, N], idt, name=f"Ti{qc}", tag=f"Ti{qc}") for qc in range(2)]
                tnr = ivp.tile([1, N], idt, name="tnr", tag="tnr")
                tni = ivp.tile([1, N], idt, name="tni", tag="tni")
                if PQ:
                    for qc in range(2):
                        for (dst, blk) in ((Tr[qc], 2 * qc), (Ti[qc], 2 * qc + 1)):
                            rowp = rs_o[blk * 128 * NW:(blk + 1) * 128 * NW] \
                                .rearrange("(p f) -> p f", p=128)
                            dmae.dma_start(dst[:, 0:NQ], rowp[:, 0:NQ])
                            dmae.dma_start(dst[:, NQ:N], rowp[:, NE:NE + 255])
                else:
                    for qc in range(2):
                        base = qc * 2 * 128 * N
                        dmae.dma_start(Tr[qc][:], rs_o[base:base + 128 * N].rearrange("(p f) -> p f", p=128))
                        dmae.dma_start(Ti[qc][:], rs_o[base + 128 * N:base + 2 * 128 * N].rearrange("(p f) -> p f", p=128))
                nyb = (4 * 128 * NW) if PQ else (2 * 2 * 128 * N)
                dmae.dma_start(tnr[:, 0:NQ], rs_o[nyb:nyb + NQ].rearrange("(p f) -> p f", p=1))
                dmae.dma_start(tni[:, 0:NQ], rs_o[nyb + N:nyb + N + NQ].rearrange("(p f) -> p f", p=1))
                # nyq fixup: T[256, 257:512] = T[256, 1:256]
                nc.vector.scalar_tensor_tensor(tnr[:, NQ:N], tnr[:, 1:256], 0.0, tnr[:, 1:256], op0=mult, op1=add)
                nc.vector.scalar_tensor_tensor(tni[:, NQ:N], tni[:, 1:256], 0.0, tni[:, 1:256], op0=mult, op1=add)

                # inv stage 1: R[j, n] per j-chunk; slices [0:128],[128:256],[257:385],[385:512], plus j=256 strip
                jsl = [(slice(0, 128), 128, True), (slice(128, 256), 128, True),
                       (slice(257, 384), 127, False), (slice(384, 512), 128, False)]
                Rr, Ri = [], []
                for (js, mw, plus) in jsl:
                    prr = psB.tile([mw, N], f32, name="prr", tag="ps2")
                    pri = psB.tile([mw, N], f32, name="pri", tag="ps2")
                    for qk in range(3):       # q chunks: 128,128,1(nyq strip)
                        st, sp = (qk == 0), (qk == 2)
                        if qk < 2:
                            lr, li = Tr[qk][:, js], Ti[qk][:, js]
                        else:
                            lr, li = tnr[:, js], tni[:, js]
                        # Rr = Tr.Acos -/+ Ti.Asin ; sign folded via const choice
                        nc.tensor.matmul(prr[:], lhsT=lr, rhs=Ac[qk][:], start=st, stop=False)
                        nc.tensor.matmul(prr[:], lhsT=li, rhs=(An if plus else As)[qk][:], start=False, stop=sp)
                        # Ri = Tr.(+/-Asin) + Ti.Acos
                        nc.tensor.matmul(pri[:], lhsT=lr, rhs=(As if plus else An)[qk][:], start=st, stop=False)
                        nc.tensor.matmul(pri[:], lhsT=li, rhs=Ac[qk][:], start=False, stop=sp)
                    rr = ivp.tile([mw, N], idt, name="rr", tag=f"rr{js.start}")
                    ri = ivp.tile([mw, N], idt, name="ri", tag=f"ri{js.start}")
                    nc.scalar.copy(rr[:], prr[:])
                    nc.scalar.copy(ri[:], pri[:])
                    Rr.append(rr)
                    Ri.append(ri)
                # j=256 column strip (uses '+' signs)
                p6r = psN.tile([1, N], f32, name="p6r", tag="psn")
                p6i = psN.tile([1, N], f32, name="p6i", tag="psn")
                for qk in range(3):
                    st, sp = (qk == 0), (qk == 2)
                    if qk < 2:
                        lr, li = Tr[qk][:, 256:257], Ti[qk][:, 256:257]
                    else:
                        lr, li = tnr[:, 256:257], tni[:, 256:257]
                    nc.tensor.matmul(p6r[:], lhsT=lr, rhs=Ac[qk][:], start=st, stop=False)
                    nc.tensor.matmul(p6r[:], lhsT=li, rhs=An[qk][:], start=False, stop=sp)
                    nc.tensor.matmul(p6i[:], lhsT=lr, rhs=As[qk][:], start=st, stop=False)
                    nc.tensor.matmul(p6i[:], lhsT=li, rhs=Ac[qk][:], start=False, stop=sp)
                r6r = ivp.tile([1, N], idt, name="r6r", tag="r6r")
                r6i = ivp.tile([1, N], idt, name="r6i", tag="r6i")
                nc.scalar.copy(r6r[:], p6r[:])
                nc.scalar.copy(r6i[:], p6i[:])

                # inv stage 2: y[r, n] = sum_j Bcos[j,r].Rr[j,n] + Bsin[j,r].Ri[j,n]
                yt = tp2.tile([128, 4 * N], f32, name="yt", tag="yt", bufs=1)
                for rc in range(4):
                    rs = slice(rc * 128, (rc + 1) * 128)
                    py = psB.tile([128, N], f32, name="py", tag="ps2")
                    kws = [128, 128, 127, 128]
                    for ji in range(5):
                        st = (ji == 0)
                        if ji < 4:
                            kw = kws[ji]
                            nc.tensor.matmul(py[:], lhsT=Bc[ji][0:kw, rs], rhs=Rr[ji][:], start=st, stop=False)
                            nc.tensor.matmul(py[:], lhsT=Bs[ji][0:kw, rs], rhs=Ri[ji][:], start=False, stop=False)
                        else:
                            nc.tensor.matmul(py[:], lhsT=Bc[4][0:1, rs], rhs=r6r[:], start=False, stop=False)
                            nc.tensor.matmul(py[:], lhsT=Bs[4][0:1, rs], rhs=r6i[:], start=False, stop=True)
                    nc.scalar.copy(yt[:, rc * N:(rc + 1) * N], py[:])
                nc.sync.dma_start(y_out.rearrange("(k p) f -> p k f", k=4),
                                  yt.rearrange("p (k f) -> p k f", k=4))

            for _rep in range(repeat):
                pp = _rep % 2
                # ---- phase 1: kernel channels ----
                for c in range(CL):
                    forward_image(kd_in[c], "k", c)

                # ---- phase 2: batches ----
                for b in range(NB):
                    # acc layout per partition: [qc0 re | qc0 im | qc1 re | qc1 im]
                    acc = ap.tile([128, (4 * NW) if PQ else (4 * N)], kdt,
                                  name="acc", tag="acc")
                    xnr = ap.tile([CL, NE], f32, name="xnr", tag="xnr", bufs=1)
                    xni = ap.tile([CL, NE], f32, name="xni", tag="xni", bufs=1)
                    for c in range(CL):
                        forward_image(xs_in[b * CL + c], "x", c, acc=acc, xnyq=(xnr, xni))
                    # nyq pointwise (per-channel rows) + channel reduce via K=4 matmul
                    t1 = tp2.tile([CL, NE], f32, name="nt1", tag="nt")
                    t2 = tp2.tile([CL, NE], f32, name="nt2", tag="nt")
                    un = tp2.tile([CL, NE], f32r, name="un", tag="un")
                    vn = tp2.tile([CL, NE], f32r, name="vn", tag="un")
                    ce = nc.vector if variant == "pw_dve" else nc.gpsimd
                    ce.tensor_tensor(t1[:], xnr[:], kfnr[:], op=mult)
                    ce.tensor_tensor(t2[:], xni[:], kfni[:], op=mult)
                    ce.tensor_tensor(un[:], t1[:], t2[:], op=sub)
                    ce.tensor_tensor(t1[:], xnr[:], kfni[:], op=mult)
                    ce.tensor_tensor(t2[:], xni[:], kfnr[:], op=mult)
                    ce.tensor_tensor(vn[:], t1[:], t2[:], op=add)
                    pnr = psN.tile([1, NE], f32, name="pnr", tag="psn")
                    pni = psN.tile([1, NE], f32, name="pni", tag="psn")
                    nc.tensor.matmul(pnr[:], lhsT=o4[0][:], rhs=un[:], start=True, stop=True)
                    nc.tensor.matmul(pni[:], lhsT=o4[0][:], rhs=vn[:], start=True, stop=True)
                    snr = tp2.tile([1, NE], kdt, name="snr", tag="sn")
                    sni = tp2.tile([1, NE], kdt, name="sni", tag="sn")
                    nc.scalar.copy(snr[:], pnr[:])
                    nc.scalar.copy(sni[:], pni[:])
                    # ship batch partial to rs_in row b
                    row = rs_in[pp][b]
                    if variant != "no_pw" and PQ:
                        nc.sync.dma_start(
                            row[0:4 * 128 * NW].rearrange("(k p f) -> p k f", k=4, p=128),
                            acc.rearrange("p (k f) -> p k f", k=4))
                    elif variant != "no_pw":
                        nc.sync.dma_start(
                            row[0:4 * 128 * N].rearrange("(k p f) -> p k f", k=4, p=128),
                            acc.rearrange("p (k f) -> p k f", k=4))
                    nyb = (4 * 128 * NW) if PQ else (2 * 2 * 128 * N)
                    nc.sync.dma_start(row[nyb:nyb + NQ].rearrange("(p f) -> p f", p=1), snr[:, 0:NQ])
                    nc.sync.dma_start(row[nyb + N:nyb + N + NQ].rearrange("(p f) -> p f", p=1), sni[:, 0:NQ])

                # ---- phase 4 (pipelined): inverse for the PREVIOUS
                # iteration's RS.  Emitted BEFORE this iteration's collective
                # so its Pool-queue loads are not stuck behind the (blocking)
                # collective instruction. ----
                if _rep > 0:
                    inverse(rs_out[1 - pp])

                # ---- phase 3: reduce-scatter (overlaps next iteration) ----
                if variant != "no_rs":
                    nc.gpsimd.collective_compute(
                        "ReduceScatter", mybir.AluOpType.add,
                        replica_groups=[list(range(NCORES))],
                        ins=[rs_in[pp][:].opt()], outs=[rs_out[pp][:].opt()],
                    )
            inverse(rs_out[(repeat - 1) % 2])

    nc.compile()
    return nc


def _get_nc(repeat=1, variant="full"):
    key = f"nc{repeat}_{variant}"
    if key not in _CACHE:
        _CACHE[key] = _build_nc(repeat, variant)
    return _CACHE[key]


def _build_exec(repeat, variant="full"):
    """Build the sharded jitted executable ONCE per repeat variant.

    run_bass_kernel_spmd creates a fresh jax.jit closure per call, so every
    call re-traces and re-loads the NEFF onto the device (seconds of
    program-size-proportional overhead). We replicate its axon path here but
    keep the jitted callable alive so repeat calls hit the executable cache.
    """
    import jax
    import concourse.mybir as mybir
    from concourse import bass2jax
    from jax.experimental.shard_map import shard_map
    from jax.sharding import Mesh, PartitionSpec

    nc = _get_nc(repeat, variant)
    bass2jax.install_neuronx_cc_hook()
    assert nc.dbg_addr is None

    partition_name = nc.partition_id_tensor.name if nc.partition_id_tensor else None
    in_names, out_names, out_avals, zero_shapes = [], [], [], []
    for alloc in nc.m.functions[0].allocations:
        if not isinstance(alloc, mybir.MemoryLocationSet):
            continue
        name = alloc.memorylocations[0].name
        if alloc.kind == "ExternalInput":
            if name != partition_name:
                in_names.append(name)
        elif alloc.kind == "ExternalOutput":
            shape = tuple(alloc.tensor_shape)
            dtype = mybir.dt.np(alloc.dtype)
            out_names.append(name)
            out_avals.append(jax.core.ShapedArray(shape, dtype))
            zero_shapes.append((shape, dtype))
    n_params = len(in_names)
    all_names = list(in_names) + list(out_names)
    if partition_name is not None:
        all_names.append(partition_name)
    donate = tuple(range(n_params, n_params + len(out_names)))

    def _body(*args):
        operands = list(args)
        if partition_name is not None:
            operands.append(bass2jax.partition_id_tensor())
        outs = bass2jax._bass_exec_p.bind(
            *operands,
            out_avals=tuple(out_avals),
            in_names=tuple(all_names),
            out_names=tuple(out_names),
            lowering_input_output_aliases=(),
            sim_require_finite=True,
            sim_require_nnan=True,
            nc=nc,
        )
        return tuple(outs)

    devices = jax.devices()[:NCORES]
    mesh = Mesh(np.asarray(devices), ("core",))
    in_specs = (PartitionSpec("core"),) * (n_params + len(out_names))
    out_specs = (PartitionSpec("core"),) * len(out_names)
    sharded = jax.jit(
        shard_map(_body, mesh=mesh, in_specs=in_specs, out_specs=out_specs,
                  check_rep=False),
        donate_argnums=donate, keep_unused=True,
    )
    return dict(sharded=sharded, mesh=mesh, in_names=in_names,
                out_names=out_names, out_avals=out_avals,
                zero_shapes=zero_shapes)


def _get_exec(repeat=1, variant="full"):
    key = f"exec{repeat}_{variant}"
    if key not in _CACHE:
        _CACHE[key] = _build_exec(repeat, variant)
    return _CACHE[key]


def _const_dev(mesh, bf):
    """Constants concatenated across cores, device-placed once."""
    key = f"cdev{int(bf)}"
    if key not in _CACHE:
        import jax
        import ml_dtypes
        from jax.sharding import NamedSharding, PartitionSpec
        C = _consts()
        sh = NamedSharding(mesh, PartitionSpec("core"))
        cdev = {}
        for nm in CONST_SHAPES:
            base = C[nm]
            if bf and nm in CONST_BF:
                base = base.astype(ml_dtypes.bfloat16)
            arr = np.broadcast_to(base, (NCORES,) + base.shape)
            arr = arr.reshape(NCORES * base.shape[0], *base.shape[1:])
            cdev[nm] = jax.device_put(np.ascontiguousarray(arr), sh)
        _CACHE[key] = cdev
    return _CACHE[key]


def kernel(x, kernel, variable_kernel, loc_idx, _repeat=1, _xdev=None, _kdev=None,
           _variant="pq"):
    ex = _get_exec(_repeat, _variant)
    cdev = _const_dev(ex["mesh"], False)
    if _kdev is None:
        vk = np.asarray(variable_kernel)
        idx = np.asarray(loc_idx)
        # host scatter of relu'd values into the dense PSF (data movement only)
        kflat = np.asarray(kernel)[0].reshape(-1).copy()
        kflat[idx] = np.maximum(vk, 0.0).astype(np.float32)
        kd = kflat.reshape(NCORES * CL, N, N).astype(np.float32)
    else:
        kd = _kdev
    if _xdev is None:
        # xs concat layout: [core*NB*CL + b*CL + cl] = x[b, core*CL + cl]
        xs = np.ascontiguousarray(
            np.asarray(x).reshape(NB, NCORES, CL, N, N).transpose(1, 0, 2, 3, 4)
        ).reshape(NCORES * NB * CL, N, N).astype(np.float32)
    else:
        xs = _xdev
    feed = {"xs": xs, "kd": kd, **cdev}
    ins = [feed[nm] for nm in ex["in_names"]]
    zeros = [np.zeros((NCORES * s[0],) + s[1:], d) for (s, d) in ex["zero_shapes"]]
    out_arrs = ex["sharded"](*ins, *zeros)
    oidx = ex["out_names"].index("y")
    y = np.asarray(out_arrs[oidx]).reshape(NCORES, N, N)
    return y[:NB].astype(np.float32)


def stage_x(x):
    """Pre-place the (sharded) x input on device; returns handle for _xdev."""
    import jax
    from jax.sharding import NamedSharding, PartitionSpec
    ex = _get_exec(1)
    xs = np.ascontiguousarray(
        np.asarray(x).reshape(NB, NCORES, CL, N, N).transpose(1, 0, 2, 3, 4)
    ).reshape(NCORES * NB * CL, N, N).astype(np.float32)
    sh = NamedSharding(ex["mesh"], PartitionSpec("core"))
    return jax.device_put(xs, sh)


def stage_kd(kernel, variable_kernel, loc_idx):
    """Pre-place the scattered PSF on device; returns handle for _kdev."""
    import jax
    from jax.sharding import NamedSharding, PartitionSpec
    ex = _get_exec(1)
    kflat = np.asarray(kernel)[0].reshape(-1).copy()
    kflat[np.asarray(loc_idx)] = np.maximum(
        np.asarray(variable_kernel), 0.0).astype(np.float32)
    kd = kflat.reshape(NCORES * CL, N, N).astype(np.float32)
    sh = NamedSharding(ex["mesh"], PartitionSpec("core"))
    return jax.device_put(kd, sh)



# revision 23
# speedup vs baseline: 1.5330x; 1.5330x over previous
"""Trainium2 Bass kernel: per-channel circular conv via DFT matmuls, summed
over channels (sparse PSF kernel), 8-core channel-sharded SPMD.

out[b] = irfft2( sum_c rfft2(x[b,c]) * rfft2(scatter(relu(vk), idx)[c]) )

Sharding: each core owns 4 of 32 channels (forward FFTs + pointwise
multiply-accumulate), ReduceScatter(add) over batch gives core b the summed
spectrum of batch b, which it inverse-transforms. All FFTs are dense DFT
matmuls in float32r (full PE rate at moving-dim >= 256, even N required).

Spectra are kept transposed ("T-form", [q (0..256) x j (0..511)]) with the
m>256 half stored conjugated at its natural compute position ("P-form") so
no data reversal is ever needed - all permutations/conjugations/signs are
absorbed into host-precomputed constant matrices, including the inverse.
"""
import numpy as np

N = 512
NQ = 257
NE = 258          # even-padded 257 (fp32r matmul needs even moving dim)
NB = 8            # batches (one per core after reduce-scatter)
CL = 4            # channels per core
NC_TOT = 32
NCORES = 8
TH = 2 * np.pi / N
PB = 2 * 2 * N * 128 + 2 * N   # per-batch rs payload: 2 qchunks x 2 planes + nyq r/i
NW = 514          # even-aligned pointwise layout: a=[0:258), b=[258:514)

_CACHE = {}


def _consts():
    r = np.arange(N)
    m = np.arange(NQ)
    ang1 = TH * np.outer(r, m)
    FrT = np.zeros((N, NE), np.float32)
    FiT = np.zeros((N, NE), np.float32)
    FrT[:, :NQ] = np.cos(ang1)
    FiT[:, :NQ] = -np.sin(ang1)
    q = np.arange(256)
    ang2 = TH * np.outer(r, q)
    GrT = np.cos(ang2).astype(np.float32)
    GiT = (-np.sin(ang2)).astype(np.float32)
    GnT = -GiT
    altT = ((-1.0) ** r).astype(np.float32).reshape(N, 1)
    w = np.full(NQ, 2.0)
    w[0] = 1.0
    w[256] = 1.0
    angA = TH * np.outer(np.arange(NQ), r)
    Acos = (w[:, None] * np.cos(angA)).astype(np.float32)
    Asin = (w[:, None] * np.sin(angA)).astype(np.float32)
    Ansin = -Asin
    j = np.arange(N)
    angB = TH * np.outer(j, r)
    sgn = np.ones((N, N))
    sgn[257:, :] = ((-1.0) ** r)[None, :]
    Bcos_t = (np.cos(angB) * sgn / (N * N)).astype(np.float32)
    Bsin_t = (-np.sin(angB) * sgn / (N * N)).astype(np.float32)

    def bpack(Bm):
        out = np.zeros((640, N), np.float32)
        out[0:128] = Bm[0:128]
        out[128:256] = Bm[128:256]
        out[256:256 + 127] = Bm[257:384]
        out[384:512] = Bm[384:512]
        out[512:513] = Bm[256:257]
        return out
    Bcos = bpack(Bcos_t)
    Bsin = bpack(Bsin_t)
    ones4 = np.ones((CL, 1), np.float32)
    return dict(FrT=FrT, FiT=FiT, GrT=GrT, GiT=GiT, GnT=GnT, altT=altT,
                Acos=Acos, Asin=Asin, Ansin=Ansin, Bcos=Bcos, Bsin=Bsin,
                ones4=ones4)


CONST_SHAPES = dict(FrT=(N, NE), FiT=(N, NE), GrT=(N, 256), GiT=(N, 256),
                    GnT=(N, 256), altT=(N, 1), Acos=(NQ, N), Asin=(NQ, N),
                    Ansin=(NQ, N), Bcos=(640, N), Bsin=(640, N), ones4=(CL, 1))
# consts carried in bf16 when the kernel runs its bf16 spectral-domain path
CONST_BF = {"Acos", "Asin", "Ansin", "Bcos", "Bsin"}


def _build_nc(repeat=1, variant="full"):
    import concourse.bacc as bacc
    import concourse.mybir as mybir
    import concourse.tile as tile

    f32 = mybir.dt.float32
    f32r = mybir.dt.float32r
    mult = mybir.AluOpType.mult
    add = mybir.AluOpType.add
    sub = mybir.AluOpType.subtract

    PQ = variant in ("full", "pq", "no_rs", "no_pw")
    BF = variant in ("full", "no_rs", "no_pw")
    bf16 = mybir.dt.bfloat16
    # pointwise / rs / inverse data dtype: uniform end-to-end so no DMA ever
    # needs a cast (f32r is plain f32 storage)
    kdt = bf16 if BF else (f32r if PQ else f32)
    cdt = f32r                      # inverse-const dtype (inverse stays f32r)

    nc = bacc.Bacc("TRN2", target_bir_lowering=False, debug=False,
                   enable_asserts=False, num_devices=NCORES)
    xs_in = nc.dram_tensor("xs", [NB * CL, N, N], f32r, kind="ExternalInput")
    kd_in = nc.dram_tensor("kd", [CL, N, N], f32r, kind="ExternalInput")
    cins = {nm: nc.dram_tensor(nm, list(sh),
                               cdt if nm in CONST_BF else f32r,
                               kind="ExternalInput")
            for nm, sh in CONST_SHAPES.items()}
    y_out = nc.dram_tensor("y", [N, N], f32, kind="ExternalOutput")

    xio_bufs = 2 if (PQ and not BF) else 3
    crt_bufs = 4 if (PQ and not BF) else 6
    with tile.TileContext(nc) as tc:
        with tc.tile_pool(name="consts", bufs=1) as cp, \
             tc.tile_pool(name="kf", bufs=1) as kp, \
             tc.tile_pool(name="xio", bufs=xio_bufs) as xp, \
             tc.tile_pool(name="crt", bufs=crt_bufs) as crp, \
             tc.tile_pool(name="acc", bufs=2) as ap, \
             tc.tile_pool(name="tmp", bufs=2) as tp, \
             tc.tile_pool(name="tmp2", bufs=2) as tp2, \
             tc.tile_pool(name="inv", bufs=1) as ivp, \
             tc.tile_pool(name="psA", bufs=2, space="PSUM") as psA, \
             tc.tile_pool(name="psB", bufs=4, space="PSUM") as psB, \
             tc.tile_pool(name="psN", bufs=2, space="PSUM") as psN, \
             tc.tile_pool(name="dram", bufs=1, space="DRAM") as dp:

            # ---- load constants (chunked along partition) ----
            def load_const(nm, rows, cols):
                ts = []
                dt = cdt if nm in CONST_BF else f32r
                nch = (rows + 127) // 128
                for k in range(nch):
                    p = min(128, rows - k * 128)
                    t = cp.tile([p, cols], dt, name=f"{nm}{k}", tag=f"{nm}{k}")
                    nc.sync.dma_start(t[:], cins[nm][k * 128:k * 128 + p, :])
                    ts.append(t)
                return ts

            Fr = load_const("FrT", N, NE)
            Fi = load_const("FiT", N, NE)
            Gr = load_const("GrT", N, 256)
            Gi = load_const("GiT", N, 256)
            Gn = load_const("GnT", N, 256)
            alt = load_const("altT", N, 1)
            Ac = load_const("Acos", NQ, N)   # chunks: 128,128,1
            As = load_const("Asin", NQ, N)
            An = load_const("Ansin", NQ, N)
            Bc = load_const("Bcos", 640, N)
            Bs = load_const("Bsin", 640, N)
            o4 = load_const("ones4", CL, 1)

            # ---- persistent Kf storage ----
            KW = NW if PQ else N
            kfr = [[kp.tile([128, KW], kdt, name=f"kfr{c}_{qc}", tag=f"kfr{c}_{qc}") for qc in range(2)]
                   for c in range(CL)]
            kfi = [[kp.tile([128, KW], kdt, name=f"kfi{c}_{qc}", tag=f"kfi{c}_{qc}") for qc in range(2)]
                   for c in range(CL)]
            kfnr = kp.tile([CL, NE], f32, name="kfnr", tag="kfnr")   # nyq strips packed by channel
            kfni = kp.tile([CL, NE], f32, name="kfni", tag="kfni")

            def consume_pq(mode, c, qc, P, Q, U, V, acc):
                """ra=P+Q  rb=P-Q  ia=U-V  ib=U+V.  Layout (width NW=514):
                cols [0:258)=a-region (j 0..256 + zero pad), [258:514)=b-region
                (j 257..511 from psum cols 1..256, col 513 = pad).  All
                offsets/widths even: DVE f32->bf16 writes at odd offsets are
                silently corrupt (probed), so the odd 257-col split is out."""
                def combine(dst, x, y, op_a, op_b):
                    nc.vector.tensor_tensor(dst[:, 0:NE], x[:, 0:NE], y[:, 0:NE], op=op_a)
                    nc.vector.tensor_tensor(dst[:, NE:NW], x[:, 1:NQ], y[:, 1:NQ], op=op_b)
                if mode == "k":
                    combine(kfr[c][qc], P, Q, add, sub)
                    combine(kfi[c][qc], U, V, sub, add)
                    return
                if variant == "no_pw":
                    return
                rAB = tp.tile([128, NW], kdt, name="rAB", tag="rAB")
                iAB = tp.tile([128, NW], kdt, name="iAB", tag="iAB")
                combine(rAB, P, Q, add, sub)
                combine(iAB, U, V, sub, add)
                kr, ki = kfr[c][qc], kfi[c][qc]
                t1 = tp.tile([128, NW], kdt, name="t1", tag="t1")
                t2 = tp.tile([128, NW], kdt, name="t2", tag="t2")
                base = qc * 2 * NW
                ar = acc[:, base:base + NW]
                ai = acc[:, base + NW:base + 2 * NW]
                nc.vector.tensor_tensor(t1[:], rAB[:], kr[:], op=mult)
                nc.vector.tensor_tensor(t2[:], iAB[:], ki[:], op=mult)
                if c == 0:
                    nc.vector.tensor_tensor(ar, t1[:], t2[:], op=sub)
                else:
                    nc.vector.tensor_tensor(t1[:], t1[:], t2[:], op=sub)
                    nc.vector.tensor_tensor(ar, ar, t1[:], op=add)
                nc.vector.tensor_tensor(t1[:], rAB[:], ki[:], op=mult)
                nc.vector.tensor_tensor(t2[:], iAB[:], kr[:], op=mult)
                if c == 0:
                    nc.vector.tensor_tensor(ai, t1[:], t2[:], op=add)
                else:
                    nc.vector.tensor_tensor(t1[:], t1[:], t2[:], op=add)
                    nc.vector.tensor_tensor(ai, ai, t1[:], op=add)

            def consume_pair(mode, c, qc, pR, pI, region, acc):
                """pR/pI: the psum pair for this region. region 'a': output
                cols [0:257]; 'b': cols [257:512] (psum cols [1:256])."""
                if region == "a":
                    cols_p, cols_a = slice(0, NQ), slice(0, NQ)
                else:
                    cols_p, cols_a = slice(1, 256), slice(NQ, N)
                w = cols_p.stop - cols_p.start
                if variant == "no_pw" and mode == "x":
                    return
                if mode == "k":
                    nc.scalar.copy(kfr[c][qc][:, cols_a], pR[:, cols_p])
                    nc.scalar.copy(kfi[c][qc][:, cols_a], pI[:, cols_p])
                    return
                kr = kfr[c][qc][:, cols_a]
                ki = kfi[c][qc][:, cols_a]
                # DVE: products straight from PSUM; combines on Pool unless
                # pw_dve (keeps the Pool queue free for the collective).
                ce = nc.vector if variant == "pw_dve" else nc.gpsimd
                t1 = tp.tile([128, NQ], f32, name="t1", tag="t1")
                t2 = tp.tile([128, NQ], f32, name="t2", tag="t2")
                t3 = tp.tile([128, NQ], f32, name="t3", tag="t3")
                t4 = tp.tile([128, NQ], f32, name="t4", tag="t4")
                nc.vector.tensor_tensor(t1[:, :w], pR[:, cols_p], kr, op=mult)
                nc.vector.tensor_tensor(t2[:, :w], pI[:, cols_p], ki, op=mult)
                nc.vector.tensor_tensor(t3[:, :w], pR[:, cols_p], ki, op=mult)
                nc.vector.tensor_tensor(t4[:, :w], pI[:, cols_p], kr, op=mult)
                base = qc * 2 * N
                ar = acc[:, base + cols_a.start: base + cols_a.stop]
                ai = acc[:, base + N + cols_a.start: base + N + cols_a.stop]
                if c == 0:
                    ce.tensor_tensor(ar, t1[:, :w], t2[:, :w], op=sub)
                    ce.tensor_tensor(ai, t3[:, :w], t4[:, :w], op=add)
                else:
                    ce.tensor_tensor(t1[:, :w], t1[:, :w], t2[:, :w], op=sub)
                    ce.tensor_tensor(ar, ar, t1[:, :w], op=add)
                    ce.tensor_tensor(t3[:, :w], t3[:, :w], t4[:, :w], op=add)
                    ce.tensor_tensor(ai, ai, t3[:, :w], op=add)

            def forward_image(src, mode, c, acc=None, xnyq=None):
                """src: DRAM AP [N, N]. mode 'k' fills kf tiles for channel c;
                mode 'x' pointwise-accumulates vs kf into the acc tile."""
                xt = xp.tile([128, 4 * N], f32r, name="xt", tag="xt")
                nc.sync.dma_start(xt.rearrange("p (k f) -> p k f", k=4),
                                  src.rearrange("(k p) f -> p k f", k=4))
                # stage 1: C^T[n, m] (m in [0,257), col 257 zero)
                crt, cit = [], []
                for n in range(4):
                    pr = psA.tile([128, NE], f32, name="ps1r", tag="ps1")
                    pi = psA.tile([128, NE], f32, name="ps1i", tag="ps1")
                    for k in range(4):
                        lhs = xt[:, k * N + n * 128:k * N + (n + 1) * 128]
                        nc.tensor.matmul(pr[:], lhsT=lhs, rhs=Fr[k][:],
                                         start=(k == 0), stop=(k == 3))
                        nc.tensor.matmul(pi[:], lhsT=lhs, rhs=Fi[k][:],
                                         start=(k == 0), stop=(k == 3))
                    cr = crp.tile([128, NE], f32r, name="cr", tag="cr")
                    ci = crp.tile([128, NE], f32r, name="ci", tag="ci")
                    nc.scalar.copy(cr[:], pr[:])
                    nc.scalar.copy(ci[:], pi[:])
                    crt.append(cr)
                    cit.append(ci)

                # stage 2 per q-chunk.  PQ path: with Gn = -Gi the four
                # region outputs are ra=P+Q, rb=P-Q, ia=U-V, ib=U+V from just
                # four matmul chains (P=Gr.crt, Q=Gn.cit, U=Gr.cit, V=Gn.crt)
                # - half the matmuls, and each loaded weight feeds 2 MMs.
                if PQ:
                    for qc in range(2):
                        qs = slice(qc * 128, (qc + 1) * 128)
                        P = psB.tile([128, NE], f32, name="Pp", tag="ps2")
                        Q = psB.tile([128, NE], f32, name="Qp", tag="ps2")
                        U = psB.tile([128, NE], f32, name="Up", tag="ps2")
                        V = psB.tile([128, NE], f32, name="Vp", tag="ps2")
                        for k in range(4):
                            st, sp = (k == 0), (k == 3)
                            nc.tensor.matmul(P[:], lhsT=Gr[k][:, qs], rhs=crt[k][:], start=st, stop=sp)
                            nc.tensor.matmul(U[:], lhsT=Gr[k][:, qs], rhs=cit[k][:], start=st, stop=sp)
                            nc.tensor.matmul(Q[:], lhsT=Gn[k][:, qs], rhs=cit[k][:], start=st, stop=sp)
                            nc.tensor.matmul(V[:], lhsT=Gn[k][:, qs], rhs=crt[k][:], start=st, stop=sp)
                        # DVE can read at most one PSUM operand per op: stage
                        # Q/V through SBUF (ACT), combine P/U straight from PSUM
                        Qs = tp.tile([128, NE], f32, name="Qs", tag="Qs")
                        Vs = tp.tile([128, NE], f32, name="Vs", tag="Vs")
                        nc.scalar.copy(Qs[:], Q[:])
                        nc.scalar.copy(Vs[:], V[:])
                        consume_pq(mode, c, qc, P, Qs, U, Vs, acc)
                else:
                    for qc in range(2):
                        qs = slice(qc * 128, (qc + 1) * 128)
                        a_s = slice(0, NE)
                        b_s = slice(0, 256)
                        ra = psB.tile([128, NE], f32, name="ra", tag="ps2")
                        ia = psB.tile([128, NE], f32, name="ia", tag="ps2")
                        for k in range(4):
                            st, sp = (k == 0), (k == 3)
                            nc.tensor.matmul(ra[:], lhsT=Gr[k][:, qs], rhs=crt[k][:, a_s], start=st, stop=False)
                            nc.tensor.matmul(ia[:], lhsT=Gr[k][:, qs], rhs=cit[k][:, a_s], start=st, stop=False)
                            nc.tensor.matmul(ra[:], lhsT=Gn[k][:, qs], rhs=cit[k][:, a_s], start=False, stop=sp)
                            nc.tensor.matmul(ia[:], lhsT=Gi[k][:, qs], rhs=crt[k][:, a_s], start=False, stop=sp)
                        consume_pair(mode, c, qc, ra, ia, "a", acc)
                        rb = psB.tile([128, 256], f32, name="rb", tag="ps2")
                        ib = psB.tile([128, 256], f32, name="ib", tag="ps2")
                        for k in range(4):
                            st, sp = (k == 0), (k == 3)
                            nc.tensor.matmul(rb[:], lhsT=Gr[k][:, qs], rhs=crt[k][:, b_s], start=st, stop=False)
                            nc.tensor.matmul(ib[:], lhsT=Gr[k][:, qs], rhs=cit[k][:, b_s], start=st, stop=False)
                            nc.tensor.matmul(rb[:], lhsT=Gi[k][:, qs], rhs=cit[k][:, b_s], start=False, stop=sp)
                            nc.tensor.matmul(ib[:], lhsT=Gn[k][:, qs], rhs=crt[k][:, b_s], start=False, stop=sp)
                        consume_pair(mode, c, qc, rb, ib, "b", acc)

                # nyquist strip q=256 (j in [0,258))
                nr = psN.tile([1, NE], f32, name="nr", tag="psn")
                ni = psN.tile([1, NE], f32, name="ni", tag="psn")
                for k in range(4):
                    st, sp = (k == 0), (k == 3)
                    nc.tensor.matmul(nr[:], lhsT=alt[k][:], rhs=crt[k][:], start=st, stop=sp)
                    nc.tensor.matmul(ni[:], lhsT=alt[k][:], rhs=cit[k][:], start=st, stop=sp)
                sr = tp2.tile([1, NE], f32, name="nstr", tag="nstr")
                si = tp2.tile([1, NE], f32, name="nsti", tag="nstr")
                nc.scalar.copy(sr[:], nr[:])
                nc.scalar.copy(si[:], ni[:])
                dst = (kfnr, kfni) if mode == "k" else xnyq
                nc.sync.dma_start(dst[0][c:c + 1, :], sr[:])
                nc.sync.dma_start(dst[1][c:c + 1, :], si[:])

            # double-buffered collective staging so RS(i) overlaps compute(i+1)
            # PQ ships the padded NW layout verbatim (RS is elementwise, any
            # consistent layout sums fine); the inverse unpacks it.
            PBv = (4 * 128 * NW + 2 * N) if PQ else PB
            rs_in = [dp.tile([NCORES, PBv], kdt, name=f"rs_in{p}", tag=f"rs_in{p}")
                     for p in range(2)]
            rs_out = [dp.tile([PBv], kdt, name=f"rs_out{p}", tag=f"rs_out{p}")
                      for p in range(2)]

            def inverse(rs_o):
                if variant == "no_rs":
                    rs_o = rs_in[0][0]
                idt = f32r
                dmae = nc.gpsimd
                Tr = [ivp.tile([128, N], idt, name=f"Tr{qc}", tag=f"Tr{qc}") for qc in range(2)]
                Ti = [ivp.tile([128, N], idt, name=f"Ti{qc}", tag=f"Ti{qc}") for qc in range(2)]
                tnr = ivp.tile([1, N], idt, name="tnr", tag="tnr")
                tni = ivp.tile([1, N], idt, name="tni", tag="tni")
                if PQ:
                    for qc in range(2):
                        for (dst, blk) in ((Tr[qc], 2 * qc), (Ti[qc], 2 * qc + 1)):
                            rowp = rs_o[blk * 128 * NW:(blk + 1) * 128 * NW] \
                                .rearrange("(p f) -> p f", p=128)
                            dmae.dma_start(dst[:, 0:NQ], rowp[:, 0:NQ])
                            dmae.dma_start(dst[:, NQ:N], rowp[:, NE:NE + 255])
                else:
                    for qc in range(2):
                        base = qc * 2 * 128 * N
                        dmae.dma_start(Tr[qc][:], rs_o[base:base + 128 * N].rearrange("(p f) -> p f", p=128))
                        dmae.dma_start(Ti[qc][:], rs_o[base + 128 * N:base + 2 * 128 * N].rearrange("(p f) -> p f", p=128))
                nyb = (4 * 128 * NW) if PQ else (2 * 2 * 128 * N)
                dmae.dma_start(tnr[:, 0:NQ], rs_o[nyb:nyb + NQ].rearrange("(p f) -> p f", p=1))
                dmae.dma_start(tni[:, 0:NQ], rs_o[nyb + N:nyb + N + NQ].rearrange("(p f) -> p f", p=1))
                # nyq fixup: T[256, 257:512] = T[256, 1:256]
                nc.vector.scalar_tensor_tensor(tnr[:, NQ:N], tnr[:, 1:256], 0.0, tnr[:, 1:256], op0=mult, op1=add)
                nc.vector.scalar_tensor_tensor(tni[:, NQ:N], tni[:, 1:256], 0.0, tni[:, 1:256], op0=mult, op1=add)

                # inv stage 1: R[j, n] per j-chunk; slices [0:128],[128:256],[257:385],[385:512], plus j=256 strip
                jsl = [(slice(0, 128), 128, True), (slice(128, 256), 128, True),
                       (slice(257, 384), 127, False), (slice(384, 512), 128, False)]
                Rr, Ri = [], []
                for (js, mw, plus) in jsl:
                    prr = psB.tile([mw, N], f32, name="prr", tag="ps2")
                    pri = psB.tile([mw, N], f32, name="pri", tag="ps2")
                    for qk in range(3):       # q chunks: 128,128,1(nyq strip)
                        st, sp = (qk == 0), (qk == 2)
                        if qk < 2:
                            lr, li = Tr[qk][:, js], Ti[qk][:, js]
                        else:
                            lr, li = tnr[:, js], tni[:, js]
                        # Rr = Tr.Acos -/+ Ti.Asin ; sign folded via const choice
                        nc.tensor.matmul(prr[:], lhsT=lr, rhs=Ac[qk][:], start=st, stop=False)
                        nc.tensor.matmul(prr[:], lhsT=li, rhs=(An if plus else As)[qk][:], start=False, stop=sp)
                        # Ri = Tr.(+/-Asin) + Ti.Acos
                        nc.tensor.matmul(pri[:], lhsT=lr, rhs=(As if plus else An)[qk][:], start=st, stop=False)
                        nc.tensor.matmul(pri[:], lhsT=li, rhs=Ac[qk][:], start=False, stop=sp)
                    rr = ivp.tile([mw, N], idt, name="rr", tag=f"rr{js.start}")
                    ri = ivp.tile([mw, N], idt, name="ri", tag=f"ri{js.start}")
                    nc.scalar.copy(rr[:], prr[:])
                    nc.scalar.copy(ri[:], pri[:])
                    Rr.append(rr)
                    Ri.append(ri)
                # j=256 column strip (uses '+' signs)
                p6r = psN.tile([1, N], f32, name="p6r", tag="psn")
                p6i = psN.tile([1, N], f32, name="p6i", tag="psn")
                for qk in range(3):
                    st, sp = (qk == 0), (qk == 2)
                    if qk < 2:
                        lr, li = Tr[qk][:, 256:257], Ti[qk][:, 256:257]
                    else:
                        lr, li = tnr[:, 256:257], tni[:, 256:257]
                    nc.tensor.matmul(p6r[:], lhsT=lr, rhs=Ac[qk][:], start=st, stop=False)
                    nc.tensor.matmul(p6r[:], lhsT=li, rhs=An[qk][:], start=False, stop=sp)
                    nc.tensor.matmul(p6i[:], lhsT=lr, rhs=As[qk][:], start=st, stop=False)
                    nc.tensor.matmul(p6i[:], lhsT=li, rhs=Ac[qk][:], start=False, stop=sp)
                r6r = ivp.tile([1, N], idt, name="r6r", tag="r6r")
                r6i = ivp.tile([1, N], idt, name="r6i", tag="r6i")
                nc.scalar.copy(r6r[:], p6r[:])
                nc.scalar.copy(r6i[:], p6i[:])

                # inv stage 2: y[r, n] = sum_j Bcos[j,r].Rr[j,n] + Bsin[j,r].Ri[j,n]
                yt = tp2.tile([128, 4 * N], f32, name="yt", tag="yt", bufs=1)
                for rc in range(4):
                    rs = slice(rc * 128, (rc + 1) * 128)
                    py = psB.tile([128, N], f32, name="py", tag="ps2")
                    kws = [128, 128, 127, 128]
                    for ji in range(5):
                        st = (ji == 0)
                        if ji < 4:
                            kw = kws[ji]
                            nc.tensor.matmul(py[:], lhsT=Bc[ji][0:kw, rs], rhs=Rr[ji][:], start=st, stop=False)
                            nc.tensor.matmul(py[:], lhsT=Bs[ji][0:kw, rs], rhs=Ri[ji][:], start=False, stop=False)
                        else:
                            nc.tensor.matmul(py[:], lhsT=Bc[4][0:1, rs], rhs=r6r[:], start=False, stop=False)
                            nc.tensor.matmul(py[:], lhsT=Bs[4][0:1, rs], rhs=r6i[:], start=False, stop=True)
                    nc.scalar.copy(yt[:, rc * N:(rc + 1) * N], py[:])
                nc.sync.dma_start(y_out.rearrange("(k p) f -> p k f", k=4),
                                  yt.rearrange("p (k f) -> p k f", k=4))

            for _rep in range(repeat):
                pp = _rep % 2
                # ---- phase 1: kernel channels ----
                for c in range(CL):
                    forward_image(kd_in[c], "k", c)

                # ---- phase 2: batches ----
                for b in range(NB):
                    # acc layout per partition: [qc0 re | qc0 im | qc1 re | qc1 im]
                    acc = ap.tile([128, (4 * NW) if PQ else (4 * N)], kdt,
                                  name="acc", tag="acc")
                    xnr = ap.tile([CL, NE], f32, name="xnr", tag="xnr", bufs=1)
                    xni = ap.tile([CL, NE], f32, name="xni", tag="xni", bufs=1)
                    for c in range(CL):
                        forward_image(xs_in[b * CL + c], "x", c, acc=acc, xnyq=(xnr, xni))
                    # nyq pointwise (per-channel rows) + channel reduce via K=4 matmul
                    t1 = tp2.tile([CL, NE], f32, name="nt1", tag="nt")
                    t2 = tp2.tile([CL, NE], f32, name="nt2", tag="nt")
                    un = tp2.tile([CL, NE], f32r, name="un", tag="un")
                    vn = tp2.tile([CL, NE], f32r, name="vn", tag="un")
                    ce = nc.vector if variant == "pw_dve" else nc.gpsimd
                    ce.tensor_tensor(t1[:], xnr[:], kfnr[:], op=mult)
                    ce.tensor_tensor(t2[:], xni[:], kfni[:], op=mult)
                    ce.tensor_tensor(un[:], t1[:], t2[:], op=sub)
                    ce.tensor_tensor(t1[:], xnr[:], kfni[:], op=mult)
                    ce.tensor_tensor(t2[:], xni[:], kfnr[:], op=mult)
                    ce.tensor_tensor(vn[:], t1[:], t2[:], op=add)
                    pnr = psN.tile([1, NE], f32, name="pnr", tag="psn")
                    pni = psN.tile([1, NE], f32, name="pni", tag="psn")
                    nc.tensor.matmul(pnr[:], lhsT=o4[0][:], rhs=un[:], start=True, stop=True)
                    nc.tensor.matmul(pni[:], lhsT=o4[0][:], rhs=vn[:], start=True, stop=True)
                    snr = tp2.tile([1, NE], kdt, name="snr", tag="sn")
                    sni = tp2.tile([1, NE], kdt, name="sni", tag="sn")
                    nc.scalar.copy(snr[:], pnr[:])
                    nc.scalar.copy(sni[:], pni[:])
                    # ship batch partial to rs_in row b
                    row = rs_in[pp][b]
                    if variant != "no_pw" and PQ:
                        nc.sync.dma_start(
                            row[0:4 * 128 * NW].rearrange("(k p f) -> p k f", k=4, p=128),
                            acc.rearrange("p (k f) -> p k f", k=4))
                    elif variant != "no_pw":
                        nc.sync.dma_start(
                            row[0:4 * 128 * N].rearrange("(k p f) -> p k f", k=4, p=128),
                            acc.rearrange("p (k f) -> p k f", k=4))
                    nyb = (4 * 128 * NW) if PQ else (2 * 2 * 128 * N)
                    nc.sync.dma_start(row[nyb:nyb + NQ].rearrange("(p f) -> p f", p=1), snr[:, 0:NQ])
                    nc.sync.dma_start(row[nyb + N:nyb + N + NQ].rearrange("(p f) -> p f", p=1), sni[:, 0:NQ])

                # ---- phase 4 (pipelined): inverse for the PREVIOUS
                # iteration's RS.  Emitted BEFORE this iteration's collective
                # so its Pool-queue loads are not stuck behind the (blocking)
                # collective instruction. ----
                if _rep > 0:
                    inverse(rs_out[1 - pp])

                # ---- phase 3: reduce-scatter (overlaps next iteration) ----
                if variant != "no_rs":
                    nc.gpsimd.collective_compute(
                        "ReduceScatter", mybir.AluOpType.add,
                        replica_groups=[list(range(NCORES))],
                        ins=[rs_in[pp][:].opt()], outs=[rs_out[pp][:].opt()],
                    )
            inverse(rs_out[(repeat - 1) % 2])

    nc.compile()
    return nc


def _get_nc(repeat=1, variant="full"):
    key = f"nc{repeat}_{variant}"
    if key not in _CACHE:
        _CACHE[key] = _build_nc(repeat, variant)
    return _CACHE[key]


def _build_exec(repeat, variant="full"):
    """Build the sharded jitted executable ONCE per repeat variant.

    run_bass_kernel_spmd creates a fresh jax.jit closure per call, so every
    call re-traces and re-loads the NEFF onto the device (seconds of
    program-size-proportional overhead). We replicate its axon path here but
    keep the jitted callable alive so repeat calls hit the executable cache.
    """
    import jax
    import concourse.mybir as mybir
    from concourse import bass2jax
    from jax.experimental.shard_map import shard_map
    from jax.sharding import Mesh, PartitionSpec

    nc = _get_nc(repeat, variant)
    bass2jax.install_neuronx_cc_hook()
    assert nc.dbg_addr is None

    partition_name = nc.partition_id_tensor.name if nc.partition_id_tensor else None
    in_names, out_names, out_avals, zero_shapes = [], [], [], []
    for alloc in nc.m.functions[0].allocations:
        if not isinstance(alloc, mybir.MemoryLocationSet):
            continue
        name = alloc.memorylocations[0].name
        if alloc.kind == "ExternalInput":
            if name != partition_name:
                in_names.append(name)
        elif alloc.kind == "ExternalOutput":
            shape = tuple(alloc.tensor_shape)
            dtype = mybir.dt.np(alloc.dtype)
            out_names.append(name)
            out_avals.append(jax.core.ShapedArray(shape, dtype))
            zero_shapes.append((shape, dtype))
    n_params = len(in_names)
    all_names = list(in_names) + list(out_names)
    if partition_name is not None:
        all_names.append(partition_name)
    donate = tuple(range(n_params, n_params + len(out_names)))

    def _body(*args):
        operands = list(args)
        if partition_name is not None:
            operands.append(bass2jax.partition_id_tensor())
        outs = bass2jax._bass_exec_p.bind(
            *operands,
            out_avals=tuple(out_avals),
            in_names=tuple(all_names),
            out_names=tuple(out_names),
            lowering_input_output_aliases=(),
            sim_require_finite=True,
            sim_require_nnan=True,
            nc=nc,
        )
        return tuple(outs)

    devices = jax.devices()[:NCORES]
    mesh = Mesh(np.asarray(devices), ("core",))
    in_specs = (PartitionSpec("core"),) * (n_params + len(out_names))
    out_specs = (PartitionSpec("core"),) * len(out_names)
    sharded = jax.jit(
        shard_map(_body, mesh=mesh, in_specs=in_specs, out_specs=out_specs,
                  check_rep=False),
        donate_argnums=donate, keep_unused=True,
    )
    return dict(sharded=sharded, mesh=mesh, in_names=in_names,
                out_names=out_names, out_avals=out_avals,
                zero_shapes=zero_shapes)


def _get_exec(repeat=1, variant="full"):
    key = f"exec{repeat}_{variant}"
    if key not in _CACHE:
        _CACHE[key] = _build_exec(repeat, variant)
    return _CACHE[key]


def _const_dev(mesh, bf):
    """Constants concatenated across cores, device-placed once."""
    key = f"cdev{int(bf)}"
    if key not in _CACHE:
        import jax
        import ml_dtypes
        from jax.sharding import NamedSharding, PartitionSpec
        C = _consts()
        sh = NamedSharding(mesh, PartitionSpec("core"))
        cdev = {}
        for nm in CONST_SHAPES:
            base = C[nm]
            if bf and nm in CONST_BF:
                base = base.astype(ml_dtypes.bfloat16)
            arr = np.broadcast_to(base, (NCORES,) + base.shape)
            arr = arr.reshape(NCORES * base.shape[0], *base.shape[1:])
            cdev[nm] = jax.device_put(np.ascontiguousarray(arr), sh)
        _CACHE[key] = cdev
    return _CACHE[key]


def kernel(x, kernel, variable_kernel, loc_idx, _repeat=1, _xdev=None, _kdev=None,
           _variant="pw_dve"):
    ex = _get_exec(_repeat, _variant)
    cdev = _const_dev(ex["mesh"], False)
    if _kdev is None:
        vk = np.asarray(variable_kernel)
        idx = np.asarray(loc_idx)
        # host scatter of relu'd values into the dense PSF (data movement only)
        kflat = np.asarray(kernel)[0].reshape(-1).copy()
        kflat[idx] = np.maximum(vk, 0.0).astype(np.float32)
        kd = kflat.reshape(NCORES * CL, N, N).astype(np.float32)
    else:
        kd = _kdev
    if _xdev is None:
        # xs concat layout: [core*NB*CL + b*CL + cl] = x[b, core*CL + cl]
        xs = np.ascontiguousarray(
            np.asarray(x).reshape(NB, NCORES, CL, N, N).transpose(1, 0, 2, 3, 4)
        ).reshape(NCORES * NB * CL, N, N).astype(np.float32)
    else:
        xs = _xdev
    feed = {"xs": xs, "kd": kd, **cdev}
    ins = [feed[nm] for nm in ex["in_names"]]
    zeros = [np.zeros((NCORES * s[0],) + s[1:], d) for (s, d) in ex["zero_shapes"]]
    out_arrs = ex["sharded"](*ins, *zeros)
    oidx = ex["out_names"].index("y")
    y = np.asarray(out_arrs[oidx]).reshape(NCORES, N, N)
    return y[:NB].astype(np.float32)


def stage_x(x):
    """Pre-place the (sharded) x input on device; returns handle for _xdev."""
    import jax
    from jax.sharding import NamedSharding, PartitionSpec
    ex = _get_exec(1)
    xs = np.ascontiguousarray(
        np.asarray(x).reshape(NB, NCORES, CL, N, N).transpose(1, 0, 2, 3, 4)
    ).reshape(NCORES * NB * CL, N, N).astype(np.float32)
    sh = NamedSharding(ex["mesh"], PartitionSpec("core"))
    return jax.device_put(xs, sh)


def stage_kd(kernel, variable_kernel, loc_idx):
    """Pre-place the scattered PSF on device; returns handle for _kdev."""
    import jax
    from jax.sharding import NamedSharding, PartitionSpec
    ex = _get_exec(1)
    kflat = np.asarray(kernel)[0].reshape(-1).copy()
    kflat[np.asarray(loc_idx)] = np.maximum(
        np.asarray(variable_kernel), 0.0).astype(np.float32)
    kd = kflat.reshape(NCORES * CL, N, N).astype(np.float32)
    sh = NamedSharding(ex["mesh"], PartitionSpec("core"))
    return jax.device_put(kd, sh)



# revision 25
# speedup vs baseline: 2.1915x; 1.4295x over previous
"""Trainium2 Bass kernel: per-channel circular conv via DFT matmuls, summed
over channels (sparse PSF kernel), 8-core channel-sharded SPMD.

out[b] = irfft2( sum_c rfft2(x[b,c]) * rfft2(scatter(relu(vk), idx)[c]) )

Sharding: each core owns 4 of 32 channels (forward FFTs + pointwise
multiply-accumulate), ReduceScatter(add) over batch gives core b the summed
spectrum of batch b, which it inverse-transforms. All FFTs are dense DFT
matmuls in float32r (full PE rate at moving-dim >= 256, even N required).

Spectra are kept transposed ("T-form", [q (0..256) x j (0..511)]) with the
m>256 half stored conjugated at its natural compute position ("P-form") so
no data reversal is ever needed - all permutations/conjugations/signs are
absorbed into host-precomputed constant matrices, including the inverse.

Performance structure (vs the original bass_utils-per-call version):
- The jitted shard_map executable is built ONCE per program variant and
  cached; run_bass_kernel_spmd would otherwise re-trace and re-load the
  NEFF on every call (seconds of overhead that dwarf the ~1ms exec).
- Stage-2 is split into two sequential psum pairs per q-chunk so the PE
  accumulates one pair while the DVE consumes the other; weight (lhsT)
  loads are grouped so each serves two matmuls.
- All pointwise multiply-accumulate runs on the DVE ("pw_dve"): the
  gpsimd queue must stay empty because the collective instruction blocks
  it until the RS completes, which would serialize RS with compute.
- The inverse transform is software-pipelined one iteration behind the
  ReduceScatter (double-buffered rs_in/rs_out) and emitted BEFORE the
  collective, so in the repeat loop the RS overlaps the next iteration's
  forward work and the PE never idles on the wire.
- One merged DMA per image load / per batch ship (fewer, larger
  transfers).

Variants kept for A/B timing: "pq"/"full" restructure stage 2 via
P=Gr.crt, Q=Gn.cit, U=Gr.cit, V=Gn.crt (ra=P+Q rb=P-Q ia=U-V ib=U+V,
half the stage-2 matmuls); "full" additionally runs the pointwise/RS/
inverse chain in bf16. The bf16 chain still has a correctness bug (wrong
by ~1e3 despite all piecewise probes passing), and "pq" in f32r measures
slower than "pw_dve" (extra ACT staging + shrunk pools), so "pw_dve" is
the default.
"""
import numpy as np

N = 512
NQ = 257
NE = 258          # even-padded 257 (fp32r matmul needs even moving dim)
NB = 8            # batches (one per core after reduce-scatter)
CL = 4            # channels per core
NC_TOT = 32
NCORES = 8
TH = 2 * np.pi / N
PB = 2 * 2 * N * 128 + 2 * N   # per-batch rs payload: 2 qchunks x 2 planes + nyq r/i
NW = 514          # even-aligned pointwise layout: a=[0:258), b=[258:514)

_CACHE = {}


def _consts():
    r = np.arange(N)
    m = np.arange(NQ)
    ang1 = TH * np.outer(r, m)
    FrT = np.zeros((N, NE), np.float32)
    FiT = np.zeros((N, NE), np.float32)
    FrT[:, :NQ] = np.cos(ang1)
    FiT[:, :NQ] = -np.sin(ang1)
    q = np.arange(256)
    ang2 = TH * np.outer(r, q)
    GrT = np.cos(ang2).astype(np.float32)
    GiT = (-np.sin(ang2)).astype(np.float32)
    GnT = -GiT
    altT = ((-1.0) ** r).astype(np.float32).reshape(N, 1)
    w = np.full(NQ, 2.0)
    w[0] = 1.0
    w[256] = 1.0
    angA = TH * np.outer(np.arange(NQ), r)
    Acos = (w[:, None] * np.cos(angA)).astype(np.float32)
    Asin = (w[:, None] * np.sin(angA)).astype(np.float32)
    Ansin = -Asin
    j = np.arange(N)
    angB = TH * np.outer(j, r)
    sgn = np.ones((N, N))
    sgn[257:, :] = ((-1.0) ** r)[None, :]
    Bcos_t = (np.cos(angB) * sgn / (N * N)).astype(np.float32)
    Bsin_t = (-np.sin(angB) * sgn / (N * N)).astype(np.float32)

    def bpack(Bm):
        out = np.zeros((640, N), np.float32)
        out[0:128] = Bm[0:128]
        out[128:256] = Bm[128:256]
        out[256:256 + 127] = Bm[257:384]
        out[384:512] = Bm[384:512]
        out[512:513] = Bm[256:257]
        return out
    Bcos = bpack(Bcos_t)
    Bsin = bpack(Bsin_t)
    ones4 = np.ones((CL, 1), np.float32)
    return dict(FrT=FrT, FiT=FiT, GrT=GrT, GiT=GiT, GnT=GnT, altT=altT,
                Acos=Acos, Asin=Asin, Ansin=Ansin, Bcos=Bcos, Bsin=Bsin,
                ones4=ones4)


CONST_SHAPES = dict(FrT=(N, NE), FiT=(N, NE), GrT=(N, 256), GiT=(N, 256),
                    GnT=(N, 256), altT=(N, 1), Acos=(NQ, N), Asin=(NQ, N),
                    Ansin=(NQ, N), Bcos=(640, N), Bsin=(640, N), ones4=(CL, 1))
# consts carried in bf16 when the kernel runs its bf16 spectral-domain path
CONST_BF = {"Acos", "Asin", "Ansin", "Bcos", "Bsin"}


def _build_nc(repeat=1, variant="full"):
    import concourse.bacc as bacc
    import concourse.mybir as mybir
    import concourse.tile as tile

    f32 = mybir.dt.float32
    f32r = mybir.dt.float32r
    mult = mybir.AluOpType.mult
    add = mybir.AluOpType.add
    sub = mybir.AluOpType.subtract

    PQ = variant in ("full", "pq", "no_rs", "no_pw")
    BF = variant in ("full", "no_rs", "no_pw")
    bf16 = mybir.dt.bfloat16
    # pointwise / rs / inverse data dtype: uniform end-to-end so no DMA ever
    # needs a cast (f32r is plain f32 storage)
    kdt = bf16 if BF else (f32r if PQ else f32)
    cdt = f32r                      # inverse-const dtype (inverse stays f32r)

    nc = bacc.Bacc("TRN2", target_bir_lowering=False, debug=False,
                   enable_asserts=False, num_devices=NCORES)
    xs_in = nc.dram_tensor("xs", [NB * CL, N, N], f32r, kind="ExternalInput")
    kd_in = nc.dram_tensor("kd", [CL, N, N], f32r, kind="ExternalInput")
    cins = {nm: nc.dram_tensor(nm, list(sh),
                               cdt if nm in CONST_BF else f32r,
                               kind="ExternalInput")
            for nm, sh in CONST_SHAPES.items()}
    y_out = nc.dram_tensor("y", [N, N], f32, kind="ExternalOutput")

    xio_bufs = 2 if (PQ and not BF) else 3
    crt_bufs = 6
    with tile.TileContext(nc) as tc:
        with tc.tile_pool(name="consts", bufs=1) as cp, \
             tc.tile_pool(name="kf", bufs=1) as kp, \
             tc.tile_pool(name="xio", bufs=xio_bufs) as xp, \
             tc.tile_pool(name="crt", bufs=crt_bufs) as crp, \
             tc.tile_pool(name="acc", bufs=2) as ap, \
             tc.tile_pool(name="tmp", bufs=2) as tp, \
             tc.tile_pool(name="tmp2", bufs=2) as tp2, \
             tc.tile_pool(name="inv", bufs=1) as ivp, \
             tc.tile_pool(name="psA", bufs=2, space="PSUM") as psA, \
             tc.tile_pool(name="psB", bufs=4, space="PSUM") as psB, \
             tc.tile_pool(name="psN", bufs=2, space="PSUM") as psN, \
             tc.tile_pool(name="dram", bufs=1, space="DRAM") as dp:

            # ---- load constants (chunked along partition) ----
            def load_const(nm, rows, cols):
                ts = []
                dt = cdt if nm in CONST_BF else f32r
                nch = (rows + 127) // 128
                for k in range(nch):
                    p = min(128, rows - k * 128)
                    t = cp.tile([p, cols], dt, name=f"{nm}{k}", tag=f"{nm}{k}")
                    nc.sync.dma_start(t[:], cins[nm][k * 128:k * 128 + p, :])
                    ts.append(t)
                return ts

            Fr = load_const("FrT", N, NE)
            Fi = load_const("FiT", N, NE)
            Gr = load_const("GrT", N, 256)
            Gi = load_const("GiT", N, 256) if not PQ else None
            Gn = load_const("GnT", N, 256)
            alt = load_const("altT", N, 1)
            Ac = load_const("Acos", NQ, N)   # chunks: 128,128,1
            As = load_const("Asin", NQ, N)
            An = load_const("Ansin", NQ, N)
            Bc = load_const("Bcos", 640, N)
            Bs = load_const("Bsin", 640, N)
            o4 = load_const("ones4", CL, 1)

            # ---- persistent Kf storage ----
            KW = NW if PQ else N
            kfr = [[kp.tile([128, KW], kdt, name=f"kfr{c}_{qc}", tag=f"kfr{c}_{qc}") for qc in range(2)]
                   for c in range(CL)]
            kfi = [[kp.tile([128, KW], kdt, name=f"kfi{c}_{qc}", tag=f"kfi{c}_{qc}") for qc in range(2)]
                   for c in range(CL)]
            kfnr = kp.tile([CL, NE], f32, name="kfnr", tag="kfnr")   # nyq strips packed by channel
            kfni = kp.tile([CL, NE], f32, name="kfni", tag="kfni")

            def consume_pq(mode, c, qc, P, Q, U, V, acc):
                """ra=P+Q  rb=P-Q  ia=U-V  ib=U+V.  Layout (width NW=514):
                cols [0:258)=a-region (j 0..256 + zero pad), [258:514)=b-region
                (j 257..511 from psum cols 1..256, col 513 = pad).  All
                offsets/widths even: DVE f32->bf16 writes at odd offsets are
                silently corrupt (probed), so the odd 257-col split is out."""
                def combine(dst, x, y, op_a, op_b):
                    nc.vector.tensor_tensor(dst[:, 0:NE], x[:, 0:NE], y[:, 0:NE], op=op_a)
                    nc.vector.tensor_tensor(dst[:, NE:NW], x[:, 1:NQ], y[:, 1:NQ], op=op_b)
                if mode == "k":
                    combine(kfr[c][qc], P, Q, add, sub)
                    combine(kfi[c][qc], U, V, sub, add)
                    return
                if variant == "no_pw":
                    return
                rAB = tp.tile([128, NW], kdt, name="rAB", tag="rAB")
                iAB = tp.tile([128, NW], kdt, name="iAB", tag="iAB")
                combine(rAB, P, Q, add, sub)
                combine(iAB, U, V, sub, add)
                kr, ki = kfr[c][qc], kfi[c][qc]
                t1 = tp.tile([128, NW], kdt, name="t1", tag="t1")
                t2 = tp.tile([128, NW], kdt, name="t2", tag="t2")
                base = qc * 2 * NW
                ar = acc[:, base:base + NW]
                ai = acc[:, base + NW:base + 2 * NW]
                nc.vector.tensor_tensor(t1[:], rAB[:], kr[:], op=mult)
                nc.vector.tensor_tensor(t2[:], iAB[:], ki[:], op=mult)
                if c == 0:
                    nc.vector.tensor_tensor(ar, t1[:], t2[:], op=sub)
                else:
                    nc.vector.tensor_tensor(t1[:], t1[:], t2[:], op=sub)
                    nc.vector.tensor_tensor(ar, ar, t1[:], op=add)
                nc.vector.tensor_tensor(t1[:], rAB[:], ki[:], op=mult)
                nc.vector.tensor_tensor(t2[:], iAB[:], kr[:], op=mult)
                if c == 0:
                    nc.vector.tensor_tensor(ai, t1[:], t2[:], op=add)
                else:
                    nc.vector.tensor_tensor(t1[:], t1[:], t2[:], op=add)
                    nc.vector.tensor_tensor(ai, ai, t1[:], op=add)

            def consume_pair(mode, c, qc, pR, pI, region, acc):
                """pR/pI: the psum pair for this region. region 'a': output
                cols [0:257]; 'b': cols [257:512] (psum cols [1:256])."""
                if region == "a":
                    cols_p, cols_a = slice(0, NQ), slice(0, NQ)
                else:
                    cols_p, cols_a = slice(1, 256), slice(NQ, N)
                w = cols_p.stop - cols_p.start
                if variant == "no_pw" and mode == "x":
                    return
                if mode == "k":
                    nc.scalar.copy(kfr[c][qc][:, cols_a], pR[:, cols_p])
                    nc.scalar.copy(kfi[c][qc][:, cols_a], pI[:, cols_p])
                    return
                kr = kfr[c][qc][:, cols_a]
                ki = kfi[c][qc][:, cols_a]
                # DVE: products straight from PSUM; combines on Pool unless
                # pw_dve (keeps the Pool queue free for the collective).
                ce = nc.vector if variant == "pw_dve" else nc.gpsimd
                t1 = tp.tile([128, NQ], f32, name="t1", tag="t1")
                t2 = tp.tile([128, NQ], f32, name="t2", tag="t2")
                t3 = tp.tile([128, NQ], f32, name="t3", tag="t3")
                t4 = tp.tile([128, NQ], f32, name="t4", tag="t4")
                nc.vector.tensor_tensor(t1[:, :w], pR[:, cols_p], kr, op=mult)
                nc.vector.tensor_tensor(t2[:, :w], pI[:, cols_p], ki, op=mult)
                nc.vector.tensor_tensor(t3[:, :w], pR[:, cols_p], ki, op=mult)
                nc.vector.tensor_tensor(t4[:, :w], pI[:, cols_p], kr, op=mult)
                base = qc * 2 * N
                ar = acc[:, base + cols_a.start: base + cols_a.stop]
                ai = acc[:, base + N + cols_a.start: base + N + cols_a.stop]
                if c == 0:
                    ce.tensor_tensor(ar, t1[:, :w], t2[:, :w], op=sub)
                    ce.tensor_tensor(ai, t3[:, :w], t4[:, :w], op=add)
                else:
                    ce.tensor_tensor(t1[:, :w], t1[:, :w], t2[:, :w], op=sub)
                    ce.tensor_tensor(ar, ar, t1[:, :w], op=add)
                    ce.tensor_tensor(t3[:, :w], t3[:, :w], t4[:, :w], op=add)
                    ce.tensor_tensor(ai, ai, t3[:, :w], op=add)

            def forward_image(src, mode, c, acc=None, xnyq=None):
                """src: DRAM AP [N, N]. mode 'k' fills kf tiles for channel c;
                mode 'x' pointwise-accumulates vs kf into the acc tile."""
                xt = xp.tile([128, 4 * N], f32r, name="xt", tag="xt")
                nc.sync.dma_start(xt.rearrange("p (k f) -> p k f", k=4),
                                  src.rearrange("(k p) f -> p k f", k=4))
                # stage 1: C^T[n, m] (m in [0,257), col 257 zero)
                crt, cit = [], []
                for n in range(4):
                    pr = psA.tile([128, NE], f32, name="ps1r", tag="ps1")
                    pi = psA.tile([128, NE], f32, name="ps1i", tag="ps1")
                    for k in range(4):
                        lhs = xt[:, k * N + n * 128:k * N + (n + 1) * 128]
                        nc.tensor.matmul(pr[:], lhsT=lhs, rhs=Fr[k][:],
                                         start=(k == 0), stop=(k == 3))
                        nc.tensor.matmul(pi[:], lhsT=lhs, rhs=Fi[k][:],
                                         start=(k == 0), stop=(k == 3))
                    cr = crp.tile([128, NE], f32r, name="cr", tag="cr")
                    ci = crp.tile([128, NE], f32r, name="ci", tag="ci")
                    nc.scalar.copy(cr[:], pr[:])
                    nc.scalar.copy(ci[:], pi[:])
                    crt.append(cr)
                    cit.append(ci)

                # stage 2 per q-chunk.  PQ path: with Gn = -Gi the four
                # region outputs are ra=P+Q, rb=P-Q, ia=U-V, ib=U+V from just
                # four matmul chains (P=Gr.crt, Q=Gn.cit, U=Gr.cit, V=Gn.crt)
                # - half the matmuls, and each loaded weight feeds 2 MMs.
                if PQ:
                    for qc in range(2):
                        qs = slice(qc * 128, (qc + 1) * 128)
                        P = psB.tile([128, NE], f32, name="Pp", tag="ps2")
                        Q = psB.tile([128, NE], f32, name="Qp", tag="ps2")
                        U = psB.tile([128, NE], f32, name="Up", tag="ps2")
                        V = psB.tile([128, NE], f32, name="Vp", tag="ps2")
                        for k in range(4):
                            st, sp = (k == 0), (k == 3)
                            nc.tensor.matmul(P[:], lhsT=Gr[k][:, qs], rhs=crt[k][:], start=st, stop=sp)
                            nc.tensor.matmul(U[:], lhsT=Gr[k][:, qs], rhs=cit[k][:], start=st, stop=sp)
                            nc.tensor.matmul(Q[:], lhsT=Gn[k][:, qs], rhs=cit[k][:], start=st, stop=sp)
                            nc.tensor.matmul(V[:], lhsT=Gn[k][:, qs], rhs=crt[k][:], start=st, stop=sp)
                        # DVE can read at most one PSUM operand per op: stage
                        # Q/V through SBUF (ACT), combine P/U straight from PSUM
                        Qs = tp.tile([128, NE], f32, name="Qs", tag="Qs")
                        Vs = tp.tile([128, NE], f32, name="Vs", tag="Vs")
                        nc.scalar.copy(Qs[:], Q[:])
                        nc.scalar.copy(Vs[:], V[:])
                        consume_pq(mode, c, qc, P, Qs, U, Vs, acc)
                else:
                    for qc in range(2):
                        qs = slice(qc * 128, (qc + 1) * 128)
                        a_s = slice(0, NE)
                        b_s = slice(0, 256)
                        ra = psB.tile([128, NE], f32, name="ra", tag="ps2")
                        ia = psB.tile([128, NE], f32, name="ia", tag="ps2")
                        for k in range(4):
                            st, sp = (k == 0), (k == 3)
                            nc.tensor.matmul(ra[:], lhsT=Gr[k][:, qs], rhs=crt[k][:, a_s], start=st, stop=False)
                            nc.tensor.matmul(ia[:], lhsT=Gr[k][:, qs], rhs=cit[k][:, a_s], start=st, stop=False)
                            nc.tensor.matmul(ra[:], lhsT=Gn[k][:, qs], rhs=cit[k][:, a_s], start=False, stop=sp)
                            nc.tensor.matmul(ia[:], lhsT=Gi[k][:, qs], rhs=crt[k][:, a_s], start=False, stop=sp)
                        consume_pair(mode, c, qc, ra, ia, "a", acc)
                        rb = psB.tile([128, 256], f32, name="rb", tag="ps2")
                        ib = psB.tile([128, 256], f32, name="ib", tag="ps2")
                        for k in range(4):
                            st, sp = (k == 0), (k == 3)
                            nc.tensor.matmul(rb[:], lhsT=Gr[k][:, qs], rhs=crt[k][:, b_s], start=st, stop=False)
                            nc.tensor.matmul(ib[:], lhsT=Gr[k][:, qs], rhs=cit[k][:, b_s], start=st, stop=False)
                            nc.tensor.matmul(rb[:], lhsT=Gi[k][:, qs], rhs=cit[k][:, b_s], start=False, stop=sp)
                            nc.tensor.matmul(ib[:], lhsT=Gn[k][:, qs], rhs=crt[k][:, b_s], start=False, stop=sp)
                        consume_pair(mode, c, qc, rb, ib, "b", acc)

                # nyquist strip q=256 (j in [0,258))
                nr = psN.tile([1, NE], f32, name="nr", tag="psn")
                ni = psN.tile([1, NE], f32, name="ni", tag="psn")
                for k in range(4):
                    st, sp = (k == 0), (k == 3)
                    nc.tensor.matmul(nr[:], lhsT=alt[k][:], rhs=crt[k][:], start=st, stop=sp)
                    nc.tensor.matmul(ni[:], lhsT=alt[k][:], rhs=cit[k][:], start=st, stop=sp)
                sr = tp2.tile([1, NE], f32, name="nstr", tag="nstr")
                si = tp2.tile([1, NE], f32, name="nsti", tag="nstr")
                nc.scalar.copy(sr[:], nr[:])
                nc.scalar.copy(si[:], ni[:])
                dst = (kfnr, kfni) if mode == "k" else xnyq
                nc.sync.dma_start(dst[0][c:c + 1, :], sr[:])
                nc.sync.dma_start(dst[1][c:c + 1, :], si[:])

            # double-buffered collective staging so RS(i) overlaps compute(i+1)
            # PQ ships the padded NW layout verbatim (RS is elementwise, any
            # consistent layout sums fine); the inverse unpacks it.
            PBv = (4 * 128 * NW + 2 * N) if PQ else PB
            rs_in = [dp.tile([NCORES, PBv], kdt, name=f"rs_in{p}", tag=f"rs_in{p}")
                     for p in range(2)]
            rs_out = [dp.tile([PBv], kdt, name=f"rs_out{p}", tag=f"rs_out{p}")
                      for p in range(2)]

            def inverse(rs_o):
                if variant == "no_rs":
                    rs_o = rs_in[0][0]
                idt = f32r
                dmae = nc.gpsimd
                Tr = [ivp.tile([128, N], idt, name=f"Tr{qc}", tag=f"Tr{qc}") for qc in range(2)]
                Ti = [ivp.tile([128, N], idt, name=f"Ti{qc}", tag=f"Ti{qc}") for qc in range(2)]
                tnr = ivp.tile([1, N], idt, name="tnr", tag="tnr")
                tni = ivp.tile([1, N], idt, name="tni", tag="tni")
                if PQ:
                    for qc in range(2):
                        for (dst, blk) in ((Tr[qc], 2 * qc), (Ti[qc], 2 * qc + 1)):
                            rowp = rs_o[blk * 128 * NW:(blk + 1) * 128 * NW] \
                                .rearrange("(p f) -> p f", p=128)
                            dmae.dma_start(dst[:, 0:NQ], rowp[:, 0:NQ])
                            dmae.dma_start(dst[:, NQ:N], rowp[:, NE:NE + 255])
                else:
                    for qc in range(2):
                        base = qc * 2 * 128 * N
                        dmae.dma_start(Tr[qc][:], rs_o[base:base + 128 * N].rearrange("(p f) -> p f", p=128))
                        dmae.dma_start(Ti[qc][:], rs_o[base + 128 * N:base + 2 * 128 * N].rearrange("(p f) -> p f", p=128))
                nyb = (4 * 128 * NW) if PQ else (2 * 2 * 128 * N)
                dmae.dma_start(tnr[:, 0:NQ], rs_o[nyb:nyb + NQ].rearrange("(p f) -> p f", p=1))
                dmae.dma_start(tni[:, 0:NQ], rs_o[nyb + N:nyb + N + NQ].rearrange("(p f) -> p f", p=1))
                # nyq fixup: T[256, 257:512] = T[256, 1:256]
                nc.vector.scalar_tensor_tensor(tnr[:, NQ:N], tnr[:, 1:256], 0.0, tnr[:, 1:256], op0=mult, op1=add)
                nc.vector.scalar_tensor_tensor(tni[:, NQ:N], tni[:, 1:256], 0.0, tni[:, 1:256], op0=mult, op1=add)

                # inv stage 1: R[j, n] per j-chunk; slices [0:128],[128:256],[257:385],[385:512], plus j=256 strip
                jsl = [(slice(0, 128), 128, True), (slice(128, 256), 128, True),
                       (slice(257, 384), 127, False), (slice(384, 512), 128, False)]
                Rr, Ri = [], []
                for (js, mw, plus) in jsl:
                    prr = psB.tile([mw, N], f32, name="prr", tag="ps2")
                    pri = psB.tile([mw, N], f32, name="pri", tag="ps2")
                    for qk in range(3):       # q chunks: 128,128,1(nyq strip)
                        st, sp = (qk == 0), (qk == 2)
                        if qk < 2:
                            lr, li = Tr[qk][:, js], Ti[qk][:, js]
                        else:
                            lr, li = tnr[:, js], tni[:, js]
                        # Rr = Tr.Acos -/+ Ti.Asin ; sign folded via const choice
                        nc.tensor.matmul(prr[:], lhsT=lr, rhs=Ac[qk][:], start=st, stop=False)
                        nc.tensor.matmul(prr[:], lhsT=li, rhs=(An if plus else As)[qk][:], start=False, stop=sp)
                        # Ri = Tr.(+/-Asin) + Ti.Acos
                        nc.tensor.matmul(pri[:], lhsT=lr, rhs=(As if plus else An)[qk][:], start=st, stop=False)
                        nc.tensor.matmul(pri[:], lhsT=li, rhs=Ac[qk][:], start=False, stop=sp)
                    rr = ivp.tile([mw, N], idt, name="rr", tag=f"rr{js.start}")
                    ri = ivp.tile([mw, N], idt, name="ri", tag=f"ri{js.start}")
                    nc.scalar.copy(rr[:], prr[:])
                    nc.scalar.copy(ri[:], pri[:])
                    Rr.append(rr)
                    Ri.append(ri)
                # j=256 column strip (uses '+' signs)
                p6r = psN.tile([1, N], f32, name="p6r", tag="psn")
                p6i = psN.tile([1, N], f32, name="p6i", tag="psn")
                for qk in range(3):
                    st, sp = (qk == 0), (qk == 2)
                    if qk < 2:
                        lr, li = Tr[qk][:, 256:257], Ti[qk][:, 256:257]
                    else:
                        lr, li = tnr[:, 256:257], tni[:, 256:257]
                    nc.tensor.matmul(p6r[:], lhsT=lr, rhs=Ac[qk][:], start=st, stop=False)
                    nc.tensor.matmul(p6r[:], lhsT=li, rhs=An[qk][:], start=False, stop=sp)
                    nc.tensor.matmul(p6i[:], lhsT=lr, rhs=As[qk][:], start=st, stop=False)
                    nc.tensor.matmul(p6i[:], lhsT=li, rhs=Ac[qk][:], start=False, stop=sp)
                r6r = ivp.tile([1, N], idt, name="r6r", tag="r6r")
                r6i = ivp.tile([1, N], idt, name="r6i", tag="r6i")
                nc.scalar.copy(r6r[:], p6r[:])
                nc.scalar.copy(r6i[:], p6i[:])

                # inv stage 2: y[r, n] = sum_j Bcos[j,r].Rr[j,n] + Bsin[j,r].Ri[j,n]
                yt = tp2.tile([128, 4 * N], f32, name="yt", tag="yt", bufs=1)
                for rc in range(4):
                    rs = slice(rc * 128, (rc + 1) * 128)
                    py = psB.tile([128, N], f32, name="py", tag="ps2")
                    kws = [128, 128, 127, 128]
                    for ji in range(5):
                        st = (ji == 0)
                        if ji < 4:
                            kw = kws[ji]
                            nc.tensor.matmul(py[:], lhsT=Bc[ji][0:kw, rs], rhs=Rr[ji][:], start=st, stop=False)
                            nc.tensor.matmul(py[:], lhsT=Bs[ji][0:kw, rs], rhs=Ri[ji][:], start=False, stop=False)
                        else:
                            nc.tensor.matmul(py[:], lhsT=Bc[4][0:1, rs], rhs=r6r[:], start=False, stop=False)
                            nc.tensor.matmul(py[:], lhsT=Bs[4][0:1, rs], rhs=r6i[:], start=False, stop=True)
                    nc.scalar.copy(yt[:, rc * N:(rc + 1) * N], py[:])
                nc.sync.dma_start(y_out.rearrange("(k p) f -> p k f", k=4),
                                  yt.rearrange("p (k f) -> p k f", k=4))

            for _rep in range(repeat):
                pp = _rep % 2
                # ---- phase 1: kernel channels ----
                for c in range(CL):
                    forward_image(kd_in[c], "k", c)

                # ---- phase 2: batches ----
                for b in range(NB):
                    # acc layout per partition: [qc0 re | qc0 im | qc1 re | qc1 im]
                    acc = ap.tile([128, (4 * NW) if PQ else (4 * N)], kdt,
                                  name="acc", tag="acc")
                    xnr = ap.tile([CL, NE], f32, name="xnr", tag="xnr", bufs=1)
                    xni = ap.tile([CL, NE], f32, name="xni", tag="xni", bufs=1)
                    for c in range(CL):
                        forward_image(xs_in[b * CL + c], "x", c, acc=acc, xnyq=(xnr, xni))
                    # nyq pointwise (per-channel rows) + channel reduce via K=4 matmul
                    t1 = tp2.tile([CL, NE], f32, name="nt1", tag="nt")
                    t2 = tp2.tile([CL, NE], f32, name="nt2", tag="nt")
                    un = tp2.tile([CL, NE], f32r, name="un", tag="un")
                    vn = tp2.tile([CL, NE], f32r, name="vn", tag="un")
                    ce = nc.vector if variant == "pw_dve" else nc.gpsimd
                    ce.tensor_tensor(t1[:], xnr[:], kfnr[:], op=mult)
                    ce.tensor_tensor(t2[:], xni[:], kfni[:], op=mult)
                    ce.tensor_tensor(un[:], t1[:], t2[:], op=sub)
                    ce.tensor_tensor(t1[:], xnr[:], kfni[:], op=mult)
                    ce.tensor_tensor(t2[:], xni[:], kfnr[:], op=mult)
                    ce.tensor_tensor(vn[:], t1[:], t2[:], op=add)
                    pnr = psN.tile([1, NE], f32, name="pnr", tag="psn")
                    pni = psN.tile([1, NE], f32, name="pni", tag="psn")
                    nc.tensor.matmul(pnr[:], lhsT=o4[0][:], rhs=un[:], start=True, stop=True)
                    nc.tensor.matmul(pni[:], lhsT=o4[0][:], rhs=vn[:], start=True, stop=True)
                    snr = tp2.tile([1, NE], kdt, name="snr", tag="sn")
                    sni = tp2.tile([1, NE], kdt, name="sni", tag="sn")
                    nc.scalar.copy(snr[:], pnr[:])
                    nc.scalar.copy(sni[:], pni[:])
                    # ship batch partial to rs_in row b
                    row = rs_in[pp][b]
                    if variant != "no_pw" and PQ:
                        nc.sync.dma_start(
                            row[0:4 * 128 * NW].rearrange("(k p f) -> p k f", k=4, p=128),
                            acc.rearrange("p (k f) -> p k f", k=4))
                    elif variant != "no_pw":
                        nc.sync.dma_start(
                            row[0:4 * 128 * N].rearrange("(k p f) -> p k f", k=4, p=128),
                            acc.rearrange("p (k f) -> p k f", k=4))
                    nyb = (4 * 128 * NW) if PQ else (2 * 2 * 128 * N)
                    nc.sync.dma_start(row[nyb:nyb + NQ].rearrange("(p f) -> p f", p=1), snr[:, 0:NQ])
                    nc.sync.dma_start(row[nyb + N:nyb + N + NQ].rearrange("(p f) -> p f", p=1), sni[:, 0:NQ])

                # ---- phase 4 (pipelined): inverse for the PREVIOUS
                # iteration's RS.  Emitted BEFORE this iteration's collective
                # so its Pool-queue loads are not stuck behind the (blocking)
                # collective instruction. ----
                if _rep > 0:
                    inverse(rs_out[1 - pp])

                # ---- phase 3: reduce-scatter (overlaps next iteration) ----
                if variant != "no_rs":
                    nc.gpsimd.collective_compute(
                        "ReduceScatter", mybir.AluOpType.add,
                        replica_groups=[list(range(NCORES))],
                        ins=[rs_in[pp][:].opt()], outs=[rs_out[pp][:].opt()],
                    )
            inverse(rs_out[(repeat - 1) % 2])

    nc.compile()
    return nc


def _get_nc(repeat=1, variant="full"):
    key = f"nc{repeat}_{variant}"
    if key not in _CACHE:
        _CACHE[key] = _build_nc(repeat, variant)
    return _CACHE[key]


def _build_exec(repeat, variant="full"):
    """Build the sharded jitted executable ONCE per repeat variant.

    run_bass_kernel_spmd creates a fresh jax.jit closure per call, so every
    call re-traces and re-loads the NEFF onto the device (seconds of
    program-size-proportional overhead). We replicate its axon path here but
    keep the jitted callable alive so repeat calls hit the executable cache.
    """
    import jax
    import concourse.mybir as mybir
    from concourse import bass2jax
    from jax.experimental.shard_map import shard_map
    from jax.sharding import Mesh, PartitionSpec

    nc = _get_nc(repeat, variant)
    bass2jax.install_neuronx_cc_hook()
    assert nc.dbg_addr is None

    partition_name = nc.partition_id_tensor.name if nc.partition_id_tensor else None
    in_names, out_names, out_avals, zero_shapes = [], [], [], []
    for alloc in nc.m.functions[0].allocations:
        if not isinstance(alloc, mybir.MemoryLocationSet):
            continue
        name = alloc.memorylocations[0].name
        if alloc.kind == "ExternalInput":
            if name != partition_name:
                in_names.append(name)
        elif alloc.kind == "ExternalOutput":
            shape = tuple(alloc.tensor_shape)
            dtype = mybir.dt.np(alloc.dtype)
            out_names.append(name)
            out_avals.append(jax.core.ShapedArray(shape, dtype))
            zero_shapes.append((shape, dtype))
    n_params = len(in_names)
    all_names = list(in_names) + list(out_names)
    if partition_name is not None:
        all_names.append(partition_name)
    donate = tuple(range(n_params, n_params + len(out_names)))

    def _body(*args):
        operands = list(args)
        if partition_name is not None:
            operands.append(bass2jax.partition_id_tensor())
        outs = bass2jax._bass_exec_p.bind(
            *operands,
            out_avals=tuple(out_avals),
            in_names=tuple(all_names),
            out_names=tuple(out_names),
            lowering_input_output_aliases=(),
            sim_require_finite=True,
            sim_require_nnan=True,
            nc=nc,
        )
        return tuple(outs)

    devices = jax.devices()[:NCORES]
    mesh = Mesh(np.asarray(devices), ("core",))
    in_specs = (PartitionSpec("core"),) * (n_params + len(out_names))
    out_specs = (PartitionSpec("core"),) * len(out_names)
    sharded = jax.jit(
        shard_map(_body, mesh=mesh, in_specs=in_specs, out_specs=out_specs,
                  check_rep=False),
        donate_argnums=donate, keep_unused=True,
    )
    return dict(sharded=sharded, mesh=mesh, in_names=in_names,
                out_names=out_names, out_avals=out_avals,
                zero_shapes=zero_shapes)


def _get_exec(repeat=1, variant="full"):
    key = f"exec{repeat}_{variant}"
    if key not in _CACHE:
        _CACHE[key] = _build_exec(repeat, variant)
    return _CACHE[key]


def _const_dev(mesh, bf):
    """Constants concatenated across cores, device-placed once."""
    key = f"cdev{int(bf)}"
    if key not in _CACHE:
        import jax
        import ml_dtypes
        from jax.sharding import NamedSharding, PartitionSpec
        C = _consts()
        sh = NamedSharding(mesh, PartitionSpec("core"))
        cdev = {}
        for nm in CONST_SHAPES:
            base = C[nm]
            if bf and nm in CONST_BF:
                base = base.astype(ml_dtypes.bfloat16)
            arr = np.broadcast_to(base, (NCORES,) + base.shape)
            arr = arr.reshape(NCORES * base.shape[0], *base.shape[1:])
            cdev[nm] = jax.device_put(np.ascontiguousarray(arr), sh)
        _CACHE[key] = cdev
    return _CACHE[key]


def kernel(x, kernel, variable_kernel, loc_idx, _repeat=1, _xdev=None, _kdev=None,
           _variant="pq"):
    ex = _get_exec(_repeat, _variant)
    cdev = _const_dev(ex["mesh"], False)
    if _kdev is None:
        vk = np.asarray(variable_kernel)
        idx = np.asarray(loc_idx)
        # host scatter of relu'd values into the dense PSF (data movement only)
        kflat = np.asarray(kernel)[0].reshape(-1).copy()
        kflat[idx] = np.maximum(vk, 0.0).astype(np.float32)
        kd = kflat.reshape(NCORES * CL, N, N).astype(np.float32)
    else:
        kd = _kdev
    if _xdev is None:
        # xs concat layout: [core*NB*CL + b*CL + cl] = x[b, core*CL + cl]
        xs = np.ascontiguousarray(
            np.asarray(x).reshape(NB, NCORES, CL, N, N).transpose(1, 0, 2, 3, 4)
        ).reshape(NCORES * NB * CL, N, N).astype(np.float32)
    else:
        xs = _xdev
    feed = {"xs": xs, "kd": kd, **cdev}
    ins = [feed[nm] for nm in ex["in_names"]]
    zeros = [np.zeros((NCORES * s[0],) + s[1:], d) for (s, d) in ex["zero_shapes"]]
    out_arrs = ex["sharded"](*ins, *zeros)
    oidx = ex["out_names"].index("y")
    y = np.asarray(out_arrs[oidx]).reshape(NCORES, N, N)
    return y[:NB].astype(np.float32)


def stage_x(x):
    """Pre-place the (sharded) x input on device; returns handle for _xdev."""
    import jax
    from jax.sharding import NamedSharding, PartitionSpec
    ex = _get_exec(1)
    xs = np.ascontiguousarray(
        np.asarray(x).reshape(NB, NCORES, CL, N, N).transpose(1, 0, 2, 3, 4)
    ).reshape(NCORES * NB * CL, N, N).astype(np.float32)
    sh = NamedSharding(ex["mesh"], PartitionSpec("core"))
    return jax.device_put(xs, sh)


def stage_kd(kernel, variable_kernel, loc_idx):
    """Pre-place the scattered PSF on device; returns handle for _kdev."""
    import jax
    from jax.sharding import NamedSharding, PartitionSpec
    ex = _get_exec(1)
    kflat = np.asarray(kernel)[0].reshape(-1).copy()
    kflat[np.asarray(loc_idx)] = np.maximum(
        np.asarray(variable_kernel), 0.0).astype(np.float32)
    kd = kflat.reshape(NCORES * CL, N, N).astype(np.float32)
    sh = NamedSharding(ex["mesh"], PartitionSpec("core"))
    return jax.device_put(kd, sh)

